# revision 1
# baseline (speedup 1.0000x reference)
"""Two-layer Elman RNN (B=64, S=512, EMB=512, HID=1024) on 8 TRN2 NeuronCores.

Layer-pipelined pairs: pair p = (core p, core p+4) handles batch quarter p
(16 rows). Core p runs the layer-1 scan; core p+4 runs the layer-2 scan LAG=2
chunks behind. The pair exchanges RAW h archives (fp16, feature-major) via a
pair-wise AllGather each chunk; each core then computes its own scan input
for the next chunk in a single unified "production" matmul pass
pre = SRC @ Wprod + bprod, where the per-core weight input Wprod is
[Wi1; 0] on layer-1 cores and Wi2 on layer-2 cores, and SRC is selected
per-core by an indirect row-gather from a combined DRAM buffer (layer-1
cores pick their own embedded-token slab, layer-2 cores pick the partner's
h1 chunk). All 8 cores execute an identical SPMD program — only input data
differs.

The scan step is built around the timeline cost structure: each step is ONE
PSUM accumulation group — an identity-stationary matmul injects the produced
input chunk, 64 Wh tile matmuls accumulate on top, and a single [128,128]
Tanh drains the bank into the feature-major h archive. Only two cross-engine
semaphore hops sit on the per-step critical chain; the identity matmul
depends only on chunk-level data so it executes under the previous step's
tanh. All off-scan PE work (embedding transposes, production matmuls) is
emitted in small work items BETWEEN scan steps so it fills the per-step tanh
windows instead of bunching into a serial block at chunk boundaries.
Matmuls in fp16 (fp32 PSUM accumulate), tanh and sigmoid in fp32.

The zero fixed point of h = tanh(W h + 0) makes the warm-up chunks of the
layer-2 cores (whose prologue production weights are zero) end exactly in
the correct initial state h = 0. (Assumes b2 == 0 for the warm-up, which
holds for this model family.)
"""

from contextlib import ExitStack

import numpy as np

import concourse.bass as bass
import concourse.bacc as bacc
import concourse.mybir as mybir
import concourse.tile as tile
from concourse.bass import IndirectOffsetOnAxis
from concourse.bass_utils import run_bass_kernel_spmd
from concourse.masks import make_identity

P = 128
VOCAB, EMB, HID = 50257, 512, 1024
B, S = 64, 512
NCORES = 8
NPAIR = 4
BL = B // NPAIR           # batch rows per pair = 16
KE = EMB // P             # 4
KH = HID // P             # 8
M = HID // P              # 8
CS = 32                   # scan steps per chunk
NCH = S // CS             # 16 chunks
LAG = 2                   # chunks the layer-2 scan trails the layer-1 scan
CCOL = BL * CS            # 512 token-columns per chunk
W = M * BL                # 128 columns per step block (m, b)
CW = CS * W               # 4096 columns per chunk in combined layout
HCW = CW // 2             # half-chunk columns
SBL = S * BL // P         # 64 token-column groups total

CDT = mybir.dt.float16
NP_CDT = np.float16
F32 = mybir.dt.float32
I32 = mybir.dt.int32

_BUILT = {}
REPLICA_GROUPS = [[p, p + NPAIR] for p in range(NPAIR)]


def _tb(ap, b=BL):
    return ap.rearrange("p (t b) -> p t b", b=b)


def _tmb(ap, m):
    """m-th [P, CS, BL] slice of a [P, CW] combined (t,m,b) AP."""
    return ap.rearrange("p (t mb) -> p t mb", mb=W)[:, :, m * BL:(m + 1) * BL]


def build(local_cc=False):
    """local_cc=True replaces the AllGather with an equivalent-volume local
    DMA so the collective-free program can run under TimelineSim."""
    nc = bacc.Bacc("TRN2", target_bir_lowering=False, debug=False, num_devices=NCORES)

    xgT_d = nc.dram_tensor("xgt", [P, SBL], I32, kind="ExternalInput").ap()
    emb_d = nc.dram_tensor("emb", [VOCAB, EMB], CDT, kind="ExternalInput").ap()
    whs_d = nc.dram_tensor("whs", [HID, HID], CDT, kind="ExternalInput").ap()
    wprod_d = nc.dram_tensor("wprod", [HID, HID], CDT, kind="ExternalInput").ap()
    wi1_d = nc.dram_tensor("wi1", [EMB, HID], CDT, kind="ExternalInput").ap()
    bprod_d = nc.dram_tensor("bprod", [M, P], F32, kind="ExternalInput").ap()
    sel_d = nc.dram_tensor("sel", [P, 1], I32, kind="ExternalInput").ap()
    wd_d = nc.dram_tensor("wdk", [P, KH], CDT, kind="ExternalInput").ap()
    bd_d = nc.dram_tensor("bdv", [BL], F32, kind="ExternalInput").ap()
    y_d = nc.dram_tensor("y", [BL], F32, kind="ExternalOutput").ap()

    AF = mybir.ActivationFunctionType

    with tile.TileContext(nc) as tc, ExitStack() as ctx:
        const_pool = ctx.enter_context(tc.tile_pool(name="const", bufs=1))
        wpool = ctx.enter_context(tc.tile_pool(name="weights", bufs=1))
        cpool = ctx.enter_context(tc.tile_pool(name="ccdram", bufs=3, space="DRAM"))
        gpool = ctx.enter_context(tc.tile_pool(name="gather", bufs=8))
        slpool = ctx.enter_context(tc.tile_pool(name="slab", bufs=2))
        srpool = ctx.enter_context(tc.tile_pool(name="src", bufs=2))
        prepool = ctx.enter_context(tc.tile_pool(name="pre", bufs=2))
        apool = ctx.enter_context(tc.tile_pool(name="arch", bufs=2))
        bigps = ctx.enter_context(tc.tile_pool(name="bigps", bufs=2, space="PSUM"))
        tppool = ctx.enter_context(tc.tile_pool(name="tpps", bufs=2, space="PSUM"))
        spApool = ctx.enter_context(tc.tile_pool(name="spsumA", bufs=2, space="PSUM"))
        spBpool = ctx.enter_context(tc.tile_pool(name="spsumB", bufs=2, space="PSUM"))

        ident = const_pool.tile([P, P], CDT, name="ident")
        make_identity(nc, ident[:])
        bpt = const_pool.tile([P, M], F32, name="bpt")
        nc.sync.dma_start(out=bpt[:], in_=bprod_d.rearrange("m p -> p m"))
        sel_sb = const_pool.tile([P, 1], I32, name="sel_sb")
        nc.sync.dma_start(out=sel_sb[:], in_=sel_d[:])
        wd_sb = const_pool.tile([P, KH], CDT, name="wd_sb")
        nc.sync.dma_start(out=wd_sb[:], in_=wd_d[:])
        bd_sb = const_pool.tile([P, 1], F32, name="bd_sb")
        nc.sync.dma_start(out=bd_sb[0:BL, 0:1], in_=bd_d[:])
        xgT_sb = const_pool.tile([P, SBL], I32, name="xgT_sb")
        nc.sync.dma_start(out=xgT_sb[:], in_=xgT_d[:])

        whs_sb = wpool.tile([P, KH * M * P], CDT, name="whs_sb")
        for k in range(KH):
            nc.sync.dma_start(out=whs_sb[:, k * HID:(k + 1) * HID],
                              in_=whs_d[k * P:(k + 1) * P, :])
        wprod_sb = wpool.tile([P, KH * M * P], CDT, name="wprod_sb")
        for k in range(KH):
            nc.sync.dma_start(out=wprod_sb[:, k * HID:(k + 1) * HID],
                              in_=wprod_d[k * P:(k + 1) * P, :])
        wi_sb = wpool.tile([P, KE * M * P], CDT, name="wi_sb")
        for e in range(KE):
            nc.sync.dma_start(out=wi_sb[:, e * HID:(e + 1) * HID],
                              in_=wi1_d[e * P:(e + 1) * P, :])

        def emit_gathers(tc_):
            """Start the 4 embedding gathers for token chunk tc_."""
            xes = []
            for gi in range(CCOL // P):
                g = tc_ * (CCOL // P) + gi
                xe_g = gpool.tile([P, EMB], CDT, tag="xe", name=f"xe{tc_}_{gi}_{g}")
                nc.gpsimd.indirect_dma_start(
                    out=xe_g[:], out_offset=None, in_=emb_d[:],
                    in_offset=IndirectOffsetOnAxis(ap=xgT_sb[:, g:g + 1], axis=0))
                xes.append(xe_g)
            return xes

        def slab_items(tc_, slab, xes, zero_tail):
            """Work items filling slab (t,e,b layout) from gathered rows."""
            items = []
            if zero_tail:
                def z():
                    v = slab[:].rearrange("p (t q) -> p t q", q=W)
                    nc.vector.memset(v[:, :, KE * BL:], 0.0)
                items.append(z)
            for gi in range(CCOL // P):
                for e in range(KE):
                    def tr(gi=gi, e=e):
                        pt = tppool.tile([P, P], CDT, tag="tp",
                                         name=f"tp{tc_}_{gi}_{e}")
                        nc.tensor.transpose(
                            out=pt[:], in_=xes[gi][:, e * P:(e + 1) * P],
                            identity=ident[:])
                        dst = slab[:].rearrange("p (t q) -> p t q", q=W)[
                            :, gi * 8:(gi + 1) * 8, e * BL:(e + 1) * BL]
                        nc.vector.tensor_copy(out=dst, in_=_tb(pt[:]))
                    items.append(tr)
            return items

        def prod_items(name, src, pre, wsb, nk, half):
            """Work items for pre[half] = src[half] @ w + bprod: per m, nk
            k-tile matmuls over the half-chunk's time steps, then a DVE
            scatter with per-partition bias."""
            items = []
            t0, t1 = half * (CS // 2), (half + 1) * (CS // 2)
            srcv = src[:].rearrange("p (t q) -> p t q", q=W)[:, t0:t1, :]
            hc = (CS // 2) * BL
            for m in range(M):
                ps2 = bigps.tile([P, hc], F32, tag="ps",
                                 name=f"pp{name}_{m}_{half}")
                for j in range(nk):
                    def mm(m=m, j=j, ps2=ps2):
                        nc.tensor.matmul(
                            ps2[:], lhsT=wsb[:, (j * M + m) * P:
                                            (j * M + m + 1) * P],
                            rhs=srcv[:, :, j * BL:(j + 1) * BL],
                            start=(j == 0), stop=(j == nk - 1))
                        if j == nk - 1:
                            out = pre[:].rearrange(
                                "p (t q) -> p t q", q=W)[
                                :, t0:t1, m * BL:(m + 1) * BL]
                            nc.vector.tensor_scalar_add(
                                out=out, in0=_tb(ps2[:]),
                                scalar1=bpt[:, m:m + 1])
                    items.append(mm)
            return items

        def scan_chunk(c, pre, arch_prev, items, mid_cb=None):
            """CS scan steps reading pre, writing a fresh arch; pops work
            items into the tanh windows between steps; mid_cb(arch) fires
            once the first half of the chunk's archive is complete."""
            arch = apool.tile([P, CS * W], CDT, tag="arch", name=f"arch{c}")
            HW_ = W // 2
            for t in range(CS):
                first = (c == 0 and t == 0)
                rsrc = arch if t > 0 else arch_prev
                rt = t - 1 if t > 0 else CS - 1
                # Output-half groups A (m 0..3) and B (m 4..7) in separate
                # PSUM banks, each ordered k<4 first so the A group only
                # waits on the previous step's tanhA; tanhA then overlaps
                # the PE's B-group work.
                for pool, m0 in ((spApool, 0), (spBpool, M // 2)):
                    c0 = m0 * BL
                    ps = pool.tile([P, HW_], F32, tag="sp",
                                   name=f"sp{c}_{t}_{m0}")
                    nc.tensor.matmul(ps[:], lhsT=ident[:],
                                     rhs=pre[:, t * W + c0:t * W + c0 + HW_],
                                     start=True, stop=first)
                    if not first:
                        for k in range(KH):
                            for m in range(m0, m0 + M // 2):
                                nc.tensor.matmul(
                                    ps[:, (m - m0) * BL:(m - m0 + 1) * BL],
                                    lhsT=whs_sb[:, (k * M + m) * P:
                                                (k * M + m + 1) * P],
                                    rhs=rsrc[:, rt * W + k * BL:
                                             rt * W + (k + 1) * BL],
                                    start=False,
                                    stop=(m == m0 + M // 2 - 1 and k == KH - 1))
                    nc.scalar.activation(
                        out=arch[:, t * W + c0:t * W + c0 + HW_],
                        in_=ps[:], func=AF.Tanh)
                if t == CS // 2 and mid_cb is not None:
                    mid_cb(arch)
                if t >= 6:
                    budget = 6
                    while items and budget > 0:
                        items.pop(0)()
                        budget -= 1
            for it in items:
                it()
            return arch

        # ---- Prologue: slabs + pre(0), pre(1) via the local-slab path ----
        # (wi_sb is [Wi1-tiles] on layer-1 cores, zeros on layer-2 cores, so
        # layer-2 warm-up chunks get pre = 0.)
        pres = {}
        pro_slabs = []
        for pi in range(LAG):
            xes = emit_gathers(pi)
            slab = slpool.tile([P, CW], CDT, tag="slab", name=f"slab{pi}")
            for it in slab_items(pi, slab, xes, zero_tail=True):
                it()
            pro_slabs.append(slab)
        pre0 = prepool.tile([P, CW], CDT, tag="pre", name="pre0")
        for half in range(2):
            for it in prod_items("w0", pro_slabs[0], pre0, wi_sb, KE, half):
                it()
        pres[0] = pre0

        # ---- Main pipelined loop ----
        arch_prev = None
        combs = {}
        for c in range(NCH + LAG):
            items = []
            slab = None
            # embedding slab for token chunk c+LAG (clamped; junk past the end)
            if c <= NCH - 1:
                tc_ = min(c + LAG, NCH - 1)
                xes = emit_gathers(tc_)
                slab = slpool.tile([P, CW], CDT, tag="slab", name=f"slabm{c}")
                items += slab_items(f"m{c}", slab, xes, zero_tail=False)
            # pre(1) production deferred into chunk 0's tanh windows
            if c == 0:
                pre1t = prepool.tile([P, CW], CDT, tag="pre", name="pre1")
                for half in range(2):
                    items += prod_items("w1", pro_slabs[1], pre1t, wi_sb,
                                        KE, half)
                pres[1] = pre1t
            # production of pre(c+1) from comb(c-1), per half-chunk so the
            # first half can start as soon as its SRC gather lands
            if 1 <= c <= NCH:
                comb_p = combs.pop(c - 1)
                src = srpool.tile([P, CW], CDT, tag="src", name=f"src{c}")
                pre_n = prepool.tile([P, CW], CDT, tag="pre", name=f"pre{c + 1}")
                for half in range(2):
                    nc.gpsimd.indirect_dma_start(
                        out=src[:, half * HCW:(half + 1) * HCW],
                        out_offset=None, in_=comb_p[half][:],
                        in_offset=IndirectOffsetOnAxis(ap=sel_sb[:, 0:1],
                                                       axis=0))
                    items += prod_items(f"p{c}", src, pre_n, wprod_sb, KH, half)
                pres[c + 1] = pre_n

            mid_cb = None
            if c <= NCH - 1:
                sends = [cpool.tile([P, HCW], CDT, space="DRAM",
                                    name=f"send_db{c}_{h}") for h in range(2)]
                comb = [cpool.tile([3 * P, HCW], CDT, space="DRAM",
                                   name=f"comb{c}_{h}") for h in range(2)]

                def halfio(arch, half, sends=sends, comb=comb, slab=slab):
                    h0, h1 = half * HCW, (half + 1) * HCW
                    nc.sync.dma_start(out=sends[half][:], in_=arch[:, h0:h1])
                    if local_cc:
                        nc.gpsimd.dma_start(out=comb[half][0:P, :],
                                            in_=sends[half][:])
                    else:
                        nc.gpsimd.collective_compute(
                            "AllGather", mybir.AluOpType.bypass,
                            ins=[sends[half][:]], outs=[comb[half][0:2 * P, :]],
                            replica_groups=REPLICA_GROUPS)
                    nc.sync.dma_start(out=comb[half][2 * P:3 * P, :],
                                      in_=slab[:, h0:h1])

                mid_cb = lambda arch, f=halfio: f(arch, 0)
                combs[c] = comb

            arch_prev = scan_chunk(c, pres.pop(c), arch_prev, items, mid_cb)

            if c <= NCH - 1:
                halfio(arch_prev, 1)

        # ---- head ----
        if True:
            hps = spApool.tile([BL, 1], F32, tag="sp", name="hps")
            for k in range(KH):
                last = arch_prev[:, (CS - 1) * W + k * BL:
                                 (CS - 1) * W + (k + 1) * BL]
                nc.tensor.matmul(
                    hps[:], lhsT=last,
                    rhs=wd_sb[:, k:k + 1], start=(k == 0), stop=(k == KH - 1))
            y_sb = const_pool.tile([P, 1], F32, name="y_sb")
            nc.scalar.activation(out=y_sb[0:BL, 0:1], in_=hps[:],
                                 func=AF.Sigmoid, bias=bd_sb[0:BL, 0:1])
            nc.sync.dma_start(out=y_d[:], in_=y_sb[0:BL, 0:1])

    nc.compile()
    return nc


def _prep_maps(x, emb, Wi1, Wh1, b1, Wi2, Wh2, b2, Wd, bd):
    f = NP_CDT
    x = np.asarray(x, np.int32)
    shared = {
        "emb": np.ascontiguousarray(np.asarray(emb, f)),
        "wdk": np.ascontiguousarray(np.asarray(Wd, f).reshape(KH, P).T),
        "bdv": np.ascontiguousarray(np.broadcast_to(
            np.asarray(bd, np.float32), (BL,))),
    }
    wi1_f = np.asarray(Wi1, f)
    wprod_a = np.concatenate([wi1_f, np.zeros((HID - EMB, HID), f)])
    wprod_b = np.ascontiguousarray(np.asarray(Wi2, f))
    wi1_b = np.zeros((EMB, HID), f)
    b_a = np.ascontiguousarray(np.asarray(b1, np.float32).reshape(M, P))
    b_b = np.ascontiguousarray(np.asarray(b2, np.float32).reshape(M, P))
    wh1 = np.ascontiguousarray(np.asarray(Wh1, f))
    wh2 = np.ascontiguousarray(np.asarray(Wh2, f))
    sel_a = (2 * P + np.arange(P, dtype=np.int32)).reshape(P, 1)
    sel_b = np.arange(P, dtype=np.int32).reshape(P, 1)
    in_maps = []
    for c in range(NCORES):
        p = c % NPAIR
        xs = x[p * BL:(p + 1) * BL, :]                    # [16, 512]
        xgrp = np.ascontiguousarray(xs.T).reshape(-1, P)  # (t, b) order
        role_a = c < NPAIR
        in_maps.append({
            **shared,
            "xgt": np.ascontiguousarray(xgrp.T),          # [P, SBL]
            "whs": wh1 if role_a else wh2,
            "wprod": wprod_a if role_a else wprod_b,
            "wi1": np.ascontiguousarray(wi1_f) if role_a else wi1_b,
            "bprod": b_a if role_a else b_b,
            "sel": sel_a if role_a else sel_b,
        })
    return in_maps


def kernel(x, emb, Wi1, Wh1, b1, Wi2, Wh2, b2, Wd, bd):
    if "nc" not in _BUILT:
        _BUILT["nc"] = build()
    nc = _BUILT["nc"]
    in_maps = _prep_maps(x, emb, Wi1, Wh1, b1, Wi2, Wh2, b2, Wd, bd)
    res = run_bass_kernel_spmd(nc, in_maps, list(range(NCORES)))
    kernel.last_result = res
    y = np.concatenate([np.asarray(res.results[NPAIR + p]["y"], np.float32)
                        for p in range(NPAIR)])
    return y



# revision 2
# speedup vs baseline: 1.0019x; 1.0019x over previous
"""Two-layer Elman RNN (B=64, S=512, EMB=512, HID=1024) on 8 TRN2 NeuronCores.

Layer-pipelined pairs: pair p = (core p, core p+4) handles batch quarter p
(16 rows). Core p runs the layer-1 scan; core p+4 runs the layer-2 scan LAG=2
chunks behind. The pair exchanges RAW h archives (fp16, feature-major) via a
pair-wise AllGather each chunk; each core then computes its own scan input
for the next chunk in a single unified "production" matmul pass
pre = SRC @ Wprod + bprod, where the per-core weight input Wprod is
[Wi1; 0] on layer-1 cores and Wi2 on layer-2 cores, and SRC is selected
per-core by an indirect row-gather from a combined DRAM buffer (layer-1
cores pick their own embedded-token slab, layer-2 cores pick the partner's
h1 chunk). All 8 cores execute an identical SPMD program — only input data
differs.

The scan step is built around the timeline cost structure: each step is ONE
PSUM accumulation group — an identity-stationary matmul injects the produced
input chunk, 64 Wh tile matmuls accumulate on top, and a single [128,128]
Tanh drains the bank into the feature-major h archive. Only two cross-engine
semaphore hops sit on the per-step critical chain; the identity matmul
depends only on chunk-level data so it executes under the previous step's
tanh. All off-scan PE work (embedding transposes, production matmuls) is
emitted in small work items BETWEEN scan steps so it fills the per-step tanh
windows instead of bunching into a serial block at chunk boundaries.
Matmuls in fp16 (fp32 PSUM accumulate), tanh and sigmoid in fp32.

The zero fixed point of h = tanh(W h + 0) makes the warm-up chunks of the
layer-2 cores (whose prologue production weights are zero) end exactly in
the correct initial state h = 0. (Assumes b2 == 0 for the warm-up, which
holds for this model family.)
"""

from contextlib import ExitStack

import numpy as np

import concourse.bass as bass
import concourse.bacc as bacc
import concourse.mybir as mybir
import concourse.tile as tile
from concourse.bass import IndirectOffsetOnAxis
from concourse.bass_utils import run_bass_kernel_spmd
from concourse.masks import make_identity

P = 128
VOCAB, EMB, HID = 50257, 512, 1024
B, S = 64, 512
NCORES = 8
NPAIR = 4
BL = B // NPAIR           # batch rows per pair = 16
KE = EMB // P             # 4
KH = HID // P             # 8
M = HID // P              # 8
CS = 32                   # scan steps per chunk
NCH = S // CS             # 16 chunks
LAG = 2                   # chunks the layer-2 scan trails the layer-1 scan
CCOL = BL * CS            # 512 token-columns per chunk
W = M * BL                # 128 columns per step block (m, b)
CW = CS * W               # 4096 columns per chunk in combined layout
HCW = CW // 2             # half-chunk columns
SBL = S * BL // P         # 64 token-column groups total

CDT = mybir.dt.float16
NP_CDT = np.float16
F32 = mybir.dt.float32
I32 = mybir.dt.int32

_BUILT = {}
REPLICA_GROUPS = [[p, p + NPAIR] for p in range(NPAIR)]


def _tb(ap, b=BL):
    return ap.rearrange("p (t b) -> p t b", b=b)


def _tmb(ap, m):
    """m-th [P, CS, BL] slice of a [P, CW] combined (t,m,b) AP."""
    return ap.rearrange("p (t mb) -> p t mb", mb=W)[:, :, m * BL:(m + 1) * BL]


def build(local_cc=False):
    """local_cc=True replaces the AllGather with an equivalent-volume local
    DMA so the collective-free program can run under TimelineSim."""
    nc = bacc.Bacc("TRN2", target_bir_lowering=False, debug=False, num_devices=NCORES)

    xgT_d = nc.dram_tensor("xgt", [P, SBL], I32, kind="ExternalInput").ap()
    emb_d = nc.dram_tensor("emb", [VOCAB, EMB], CDT, kind="ExternalInput").ap()
    whs_d = nc.dram_tensor("whs", [HID, HID], CDT, kind="ExternalInput").ap()
    wprod_d = nc.dram_tensor("wprod", [HID, HID], CDT, kind="ExternalInput").ap()
    wi1_d = nc.dram_tensor("wi1", [EMB, HID], CDT, kind="ExternalInput").ap()
    bprod_d = nc.dram_tensor("bprod", [M, P], F32, kind="ExternalInput").ap()
    sel_d = nc.dram_tensor("sel", [P, 1], I32, kind="ExternalInput").ap()
    wd_d = nc.dram_tensor("wdk", [P, KH], CDT, kind="ExternalInput").ap()
    bd_d = nc.dram_tensor("bdv", [BL], F32, kind="ExternalInput").ap()
    y_d = nc.dram_tensor("y", [BL], F32, kind="ExternalOutput").ap()

    AF = mybir.ActivationFunctionType

    with tile.TileContext(nc) as tc, ExitStack() as ctx:
        const_pool = ctx.enter_context(tc.tile_pool(name="const", bufs=1))
        wpool = ctx.enter_context(tc.tile_pool(name="weights", bufs=1))
        cpool = ctx.enter_context(tc.tile_pool(name="ccdram", bufs=3, space="DRAM"))
        gpool = ctx.enter_context(tc.tile_pool(name="gather", bufs=8))
        slpool = ctx.enter_context(tc.tile_pool(name="slab", bufs=2))
        srpool = ctx.enter_context(tc.tile_pool(name="src", bufs=2))
        prepool = ctx.enter_context(tc.tile_pool(name="pre", bufs=2))
        apool = ctx.enter_context(tc.tile_pool(name="arch", bufs=2))
        bigps = ctx.enter_context(tc.tile_pool(name="bigps", bufs=2, space="PSUM"))
        tppool = ctx.enter_context(tc.tile_pool(name="tpps", bufs=2, space="PSUM"))
        spApool = ctx.enter_context(tc.tile_pool(name="spsumA", bufs=2, space="PSUM"))
        spBpool = ctx.enter_context(tc.tile_pool(name="spsumB", bufs=2, space="PSUM"))

        ident = const_pool.tile([P, P], CDT, name="ident")
        make_identity(nc, ident[:])
        bpt = const_pool.tile([P, M], F32, name="bpt")
        nc.sync.dma_start(out=bpt[:], in_=bprod_d.rearrange("m p -> p m"))
        sel_sb = const_pool.tile([P, 1], I32, name="sel_sb")
        nc.sync.dma_start(out=sel_sb[:], in_=sel_d[:])
        wd_sb = const_pool.tile([P, KH], CDT, name="wd_sb")
        nc.sync.dma_start(out=wd_sb[:], in_=wd_d[:])
        bd_sb = const_pool.tile([P, 1], F32, name="bd_sb")
        nc.sync.dma_start(out=bd_sb[0:BL, 0:1], in_=bd_d[:])
        xgT_sb = const_pool.tile([P, SBL], I32, name="xgT_sb")
        nc.sync.dma_start(out=xgT_sb[:], in_=xgT_d[:])

        whs_sb = wpool.tile([P, KH * M * P], CDT, name="whs_sb")
        for k in range(KH):
            nc.sync.dma_start(out=whs_sb[:, k * HID:(k + 1) * HID],
                              in_=whs_d[k * P:(k + 1) * P, :])
        wprod_sb = wpool.tile([P, KH * M * P], CDT, name="wprod_sb")
        for k in range(KH):
            nc.sync.dma_start(out=wprod_sb[:, k * HID:(k + 1) * HID],
                              in_=wprod_d[k * P:(k + 1) * P, :])
        wi_sb = wpool.tile([P, KE * M * P], CDT, name="wi_sb")
        for e in range(KE):
            nc.sync.dma_start(out=wi_sb[:, e * HID:(e + 1) * HID],
                              in_=wi1_d[e * P:(e + 1) * P, :])

        def emit_gathers(tc_):
            """Start the 4 embedding gathers for token chunk tc_."""
            xes = []
            for gi in range(CCOL // P):
                g = tc_ * (CCOL // P) + gi
                xe_g = gpool.tile([P, EMB], CDT, tag="xe", name=f"xe{tc_}_{gi}_{g}")
                nc.gpsimd.indirect_dma_start(
                    out=xe_g[:], out_offset=None, in_=emb_d[:],
                    in_offset=IndirectOffsetOnAxis(ap=xgT_sb[:, g:g + 1], axis=0))
                xes.append(xe_g)
            return xes

        def slab_items(tc_, slab, xes, zero_tail):
            """Work items filling slab (t,e,b layout) from gathered rows."""
            items = []
            if zero_tail:
                def z():
                    v = slab[:].rearrange("p (t q) -> p t q", q=W)
                    nc.vector.memset(v[:, :, KE * BL:], 0.0)
                items.append(z)
            for gi in range(CCOL // P):
                for e in range(KE):
                    def tr(gi=gi, e=e):
                        pt = tppool.tile([P, P], CDT, tag="tp",
                                         name=f"tp{tc_}_{gi}_{e}")
                        nc.tensor.transpose(
                            out=pt[:], in_=xes[gi][:, e * P:(e + 1) * P],
                            identity=ident[:])
                        dst = slab[:].rearrange("p (t q) -> p t q", q=W)[
                            :, gi * 8:(gi + 1) * 8, e * BL:(e + 1) * BL]
                        nc.vector.tensor_copy(out=dst, in_=_tb(pt[:]))
                    items.append(tr)
            return items

        def prod_items(name, src, pre, wsb, nk, half):
            """Work items for pre[half] = src[half] @ w + bprod: per m, nk
            k-tile matmuls over the half-chunk's time steps, then a DVE
            scatter with per-partition bias."""
            items = []
            t0, t1 = half * (CS // 2), (half + 1) * (CS // 2)
            srcv = src[:].rearrange("p (t q) -> p t q", q=W)[:, t0:t1, :]
            hc = (CS // 2) * BL
            for m in range(M):
                ps2 = bigps.tile([P, hc], F32, tag="ps",
                                 name=f"pp{name}_{m}_{half}")
                for j in range(nk):
                    def mm(m=m, j=j, ps2=ps2):
                        nc.tensor.matmul(
                            ps2[:], lhsT=wsb[:, (j * M + m) * P:
                                            (j * M + m + 1) * P],
                            rhs=srcv[:, :, j * BL:(j + 1) * BL],
                            start=(j == 0), stop=(j == nk - 1))
                        if j == nk - 1:
                            out = pre[:].rearrange(
                                "p (t q) -> p t q", q=W)[
                                :, t0:t1, m * BL:(m + 1) * BL]
                            nc.vector.tensor_scalar_add(
                                out=out, in0=_tb(ps2[:]),
                                scalar1=bpt[:, m:m + 1])
                    items.append(mm)
            return items

        def scan_chunk(c, pre, arch_prev, items, mid_cb=None):
            """CS scan steps reading pre, writing a fresh arch; pops work
            items into the tanh windows between steps; mid_cb(arch) fires
            once the first half of the chunk's archive is complete."""
            arch = apool.tile([P, CS * W], CDT, tag="arch", name=f"arch{c}")
            HW_ = W // 2
            for t in range(CS):
                first = (c == 0 and t == 0)
                rsrc = arch if t > 0 else arch_prev
                rt = t - 1 if t > 0 else CS - 1
                # Output-half groups A (m 0..3) and B (m 4..7) in separate
                # PSUM banks, each ordered k<4 first so the A group only
                # waits on the previous step's tanhA; tanhA then overlaps
                # the PE's B-group work.
                for pool, m0 in ((spApool, 0), (spBpool, M // 2)):
                    c0 = m0 * BL
                    ps = pool.tile([P, HW_], F32, tag="sp",
                                   name=f"sp{c}_{t}_{m0}")
                    nc.tensor.matmul(ps[:], lhsT=ident[:],
                                     rhs=pre[:, t * W + c0:t * W + c0 + HW_],
                                     start=True, stop=first)
                    if not first:
                        for k in range(KH):
                            for m in range(m0, m0 + M // 2):
                                nc.tensor.matmul(
                                    ps[:, (m - m0) * BL:(m - m0 + 1) * BL],
                                    lhsT=whs_sb[:, (k * M + m) * P:
                                                (k * M + m + 1) * P],
                                    rhs=rsrc[:, rt * W + k * BL:
                                             rt * W + (k + 1) * BL],
                                    start=False,
                                    stop=(m == m0 + M // 2 - 1 and k == KH - 1))
                    nc.scalar.activation(
                        out=arch[:, t * W + c0:t * W + c0 + HW_],
                        in_=ps[:], func=AF.Tanh)
                if t == CS // 2 and mid_cb is not None:
                    mid_cb(arch)
                if t >= 6:
                    budget = 6
                    while items and budget > 0:
                        items.pop(0)()
                        budget -= 1
            for it in items:
                it()
            return arch

        # ---- Prologue: slabs + pre(0), pre(1) via the local-slab path ----
        # (wi_sb is [Wi1-tiles] on layer-1 cores, zeros on layer-2 cores, so
        # layer-2 warm-up chunks get pre = 0.)
        pres = {}
        pro_slabs = []
        for pi in range(LAG):
            xes = emit_gathers(pi)
            slab = slpool.tile([P, CW], CDT, tag="slab", name=f"slab{pi}")
            for it in slab_items(pi, slab, xes, zero_tail=True):
                it()
            pro_slabs.append(slab)
        pre0 = prepool.tile([P, CW], CDT, tag="pre", name="pre0")
        for half in range(2):
            for it in prod_items("w0", pro_slabs[0], pre0, wi_sb, KE, half):
                it()
        pres[0] = pre0

        # ---- Main pipelined loop ----
        arch_prev = None
        combs = {}
        for c in range(NCH + LAG):
            items = []
            slab = None
            # embedding slab for token chunk c+LAG (clamped; junk past the end)
            if c <= NCH - 1:
                tc_ = min(c + LAG, NCH - 1)
                xes = emit_gathers(tc_)
                slab = slpool.tile([P, CW], CDT, tag="slab", name=f"slabm{c}")
                items += slab_items(f"m{c}", slab, xes, zero_tail=False)
            # pre(1) production deferred into chunk 0's tanh windows
            if c == 0:
                pre1t = prepool.tile([P, CW], CDT, tag="pre", name="pre1")
                for half in range(2):
                    items += prod_items("w1", pro_slabs[1], pre1t, wi_sb,
                                        KE, half)
                pres[1] = pre1t
            # production of pre(c+1) from comb(c-1), per half-chunk so the
            # first half can start as soon as its SRC gather lands
            if 1 <= c <= NCH:
                comb_p = combs.pop(c - 1)
                src = srpool.tile([P, CW], CDT, tag="src", name=f"src{c}")
                pre_n = prepool.tile([P, CW], CDT, tag="pre", name=f"pre{c + 1}")
                for half in range(2):
                    nc.gpsimd.indirect_dma_start(
                        out=src[:, half * HCW:(half + 1) * HCW],
                        out_offset=None, in_=comb_p[half][:],
                        in_offset=IndirectOffsetOnAxis(ap=sel_sb[:, 0:1],
                                                       axis=0))
                    items += prod_items(f"p{c}", src, pre_n, wprod_sb, KH, half)
                pres[c + 1] = pre_n

            mid_cb = None
            if c <= NCH - 1:
                sends = [cpool.tile([P, HCW], CDT, space="DRAM",
                                    name=f"send_db{c}_{h}") for h in range(2)]
                comb = [cpool.tile([3 * P, HCW], CDT, space="DRAM",
                                   name=f"comb{c}_{h}") for h in range(2)]

                def halfio(arch, half, sends=sends, comb=comb, slab=slab):
                    h0, h1 = half * HCW, (half + 1) * HCW
                    nc.sync.dma_start(out=sends[half][:], in_=arch[:, h0:h1])
                    if local_cc:
                        nc.gpsimd.dma_start(out=comb[half][0:P, :],
                                            in_=sends[half][:])
                    else:
                        nc.gpsimd.collective_compute(
                            "AllGather", mybir.AluOpType.bypass,
                            ins=[sends[half][:]], outs=[comb[half][0:2 * P, :]],
                            replica_groups=REPLICA_GROUPS)
                    nc.sync.dma_start(out=comb[half][2 * P:3 * P, :],
                                      in_=slab[:, h0:h1])

                mid_cb = lambda arch, f=halfio: f(arch, 0)
                combs[c] = comb

            arch_prev = scan_chunk(c, pres.pop(c), arch_prev, items, mid_cb)

            if c <= NCH - 1:
                halfio(arch_prev, 1)

        # ---- head ----
        if True:
            hps = spApool.tile([BL, 1], F32, tag="sp", name="hps")
            for k in range(KH):
                last = arch_prev[:, (CS - 1) * W + k * BL:
                                 (CS - 1) * W + (k + 1) * BL]
                nc.tensor.matmul(
                    hps[:], lhsT=last,
                    rhs=wd_sb[:, k:k + 1], start=(k == 0), stop=(k == KH - 1))
            y_sb = const_pool.tile([P, 1], F32, name="y_sb")
            nc.scalar.activation(out=y_sb[0:BL, 0:1], in_=hps[:],
                                 func=AF.Sigmoid, bias=bd_sb[0:BL, 0:1])
            nc.sync.dma_start(out=y_d[:], in_=y_sb[0:BL, 0:1])

    from semfold import prune_redundant_self_waits
    prune_redundant_self_waits(nc)
    nc.compile()
    return nc


def _prep_maps(x, emb, Wi1, Wh1, b1, Wi2, Wh2, b2, Wd, bd):
    f = NP_CDT
    x = np.asarray(x, np.int32)
    shared = {
        "emb": np.ascontiguousarray(np.asarray(emb, f)),
        "wdk": np.ascontiguousarray(np.asarray(Wd, f).reshape(KH, P).T),
        "bdv": np.ascontiguousarray(np.broadcast_to(
            np.asarray(bd, np.float32), (BL,))),
    }
    wi1_f = np.asarray(Wi1, f)
    wprod_a = np.concatenate([wi1_f, np.zeros((HID - EMB, HID), f)])
    wprod_b = np.ascontiguousarray(np.asarray(Wi2, f))
    wi1_b = np.zeros((EMB, HID), f)
    b_a = np.ascontiguousarray(np.asarray(b1, np.float32).reshape(M, P))
    b_b = np.ascontiguousarray(np.asarray(b2, np.float32).reshape(M, P))
    wh1 = np.ascontiguousarray(np.asarray(Wh1, f))
    wh2 = np.ascontiguousarray(np.asarray(Wh2, f))
    sel_a = (2 * P + np.arange(P, dtype=np.int32)).reshape(P, 1)
    sel_b = np.arange(P, dtype=np.int32).reshape(P, 1)
    in_maps = []
    for c in range(NCORES):
        p = c % NPAIR
        xs = x[p * BL:(p + 1) * BL, :]                    # [16, 512]
        xgrp = np.ascontiguousarray(xs.T).reshape(-1, P)  # (t, b) order
        role_a = c < NPAIR
        in_maps.append({
            **shared,
            "xgt": np.ascontiguousarray(xgrp.T),          # [P, SBL]
            "whs": wh1 if role_a else wh2,
            "wprod": wprod_a if role_a else wprod_b,
            "wi1": np.ascontiguousarray(wi1_f) if role_a else wi1_b,
            "bprod": b_a if role_a else b_b,
            "sel": sel_a if role_a else sel_b,
        })
    return in_maps


def kernel(x, emb, Wi1, Wh1, b1, Wi2, Wh2, b2, Wd, bd):
    if "nc" not in _BUILT:
        _BUILT["nc"] = build()
    nc = _BUILT["nc"]
    in_maps = _prep_maps(x, emb, Wi1, Wh1, b1, Wi2, Wh2, b2, Wd, bd)
    res = run_bass_kernel_spmd(nc, in_maps, list(range(NCORES)))
    kernel.last_result = res
    y = np.concatenate([np.asarray(res.results[NPAIR + p]["y"], np.float32)
                        for p in range(NPAIR)])
    return y



# revision 3
# speedup vs baseline: 1.1952x; 1.1929x over previous
"""Two-layer Elman RNN (B=64, S=512, EMB=512, HID=1024) on 8 TRN2 NeuronCores.

Pure data-parallel layout: core c owns batch rows [8c, 8c+8) and runs BOTH
layers itself as two software-pipelined recurrence chains — the layer-1 chain
at chunk k and the layer-2 chain LAG=2 chunks behind — so no collectives and
no DRAM round-trip for h1 are needed; the L2 chain's scan input is produced
locally from the L1 arch in SBUF.

Per chain-step the whole h update is ONE PSUM accumulation group in a
chunk-sized PSUM bank: the production pass (x@Wi1 for L1, h1@Wi2 for L2,
emitted a window ahead) writes the pre-activations for all CS=8 steps of the
chunk straight into the bank (first matmul start=True zero-arms the bank;
later first-touch writes auto-zero, subsequent writes accumulate), the 64
per-step Wh tile matmuls accumulate on top, and a single [128, 64] Tanh
drains the step's columns into the fp16 h archive. The critical chain per
step is tanh -> 64 matmuls -> tanh with one cross-engine semaphore hop each
direction; prune_redundant_self_waits removes tile's redundant self-engine
tick waits so dependent instructions carry their cross-engine wait directly
instead of behind a blocking EventSemaphore. The two chains' steps
interleave, filling each chain's latency window with the other chain's work
plus production/gather/transpose items.
"""

import collections
import re
from contextlib import ExitStack

import numpy as np

import concourse.bass as bass
import concourse.bacc as bacc
import concourse.mybir as mybir
import concourse.tile as tile
from concourse.bass import IndirectOffsetOnAxis
from concourse.bass_utils import run_bass_kernel_spmd
from concourse.masks import make_identity

P = 128
VOCAB, EMB, HID = 50257, 512, 1024
B, S = 64, 512
NCORES = 8
RB = B // NCORES          # 8 batch rows per core
M = HID // P              # 8 output feature blocks
KH = HID // P             # 8 contraction tiles (hidden)
KE = EMB // P             # 4 contraction tiles (embedding)
CS = 8                    # scan steps per chunk
NCH = S // CS             # 64 chunks
LAG = 2                   # chunks the L2 chain trails the L1 chain
W = M * RB                # 64 (m, b) columns per step
CW = CS * W               # 512 columns per chunk

F16 = mybir.dt.float16
F32 = mybir.dt.float32
I32 = mybir.dt.int32
NPF = np.float16

_BUILT = {}

_TICK = re.compile(r"^(PE|Activation|DVE|Pool|SP)_\d+$")


def prune_redundant_self_waits(nc):
    """Drop waits provably satisfied by same-engine program order.

    Tile assigns every instruction a wait on its own engine's tick semaphore;
    on TRN2 each instruction may carry at most ONE wait, so any cross-engine
    dependency then gets split into a separate blocking EventSemaphore that
    serializes the sequencer behind the wait and adds a post-release decode
    to the critical chain.  A wait `S >= V` on engine E is redundant iff S is
    E's own tick sem (only E's instructions increment it, once each, in
    order) and >= V increments precede this instruction in the same block.
    DMA-completion sems (DMAHW*/SWDGE) never match the tick-sem pattern.
    """
    for fn in nc.m.functions:
        for bb in fn.blocks:
            inc_count = collections.Counter()
            for inst in bb.instructions:
                si = inst.sync_info
                eng = getattr(inst.engine, "value", str(inst.engine))
                if si is not None and si.on_wait:
                    keep = []
                    for wt in si.on_wait:
                        mt = _TICK.match(wt.ant_name or "")
                        if (
                            mt is not None
                            and mt.group(1) == eng
                            and wt.sync_type == "semaphore"
                            and wt.wait_mode == "sem-ge-imm"
                            and wt.wait_reg is None
                            and inc_count.get(wt.ant_name, 0) >= wt.wait_value
                        ):
                            continue
                        keep.append(wt)
                    if len(keep) != len(si.on_wait):
                        inst.sync_info = type(si)(
                            on_wait=keep, on_update=list(si.on_update)
                        )
                si = inst.sync_info
                if si is not None:
                    for u in si.on_update:
                        if (
                            u.sync_type == "semaphore"
                            and u.update_mode == "sem-inc"
                            and _TICK.match(u.ant_name or "")
                        ):
                            inc_count[u.ant_name] += u.update_value


MB = CS * RB              # 64 cols per m-block in the (m, t, b) chunk layout


def _mv(ap):
    """View a [P, CW] chunk AP as [P, m, tb] with tb = MB cols per m-block."""
    return ap.rearrange("p (m tb) -> p m tb", tb=MB)


def build(local_cc=False, bias1=False, bias2=False, nwin=None):
    del local_cc  # no collectives in this kernel; kept for test harness compat
    nc = bacc.Bacc("TRN2", target_bir_lowering=False, debug=False,
                   num_devices=NCORES)

    xg_d = nc.dram_tensor("xg", [W, NCH], I32, kind="ExternalInput").ap()
    emb_d = nc.dram_tensor("emb", [VOCAB, EMB], F16, kind="ExternalInput").ap()
    wh1_d = nc.dram_tensor("wh1", [HID, HID], F16, kind="ExternalInput").ap()
    wh2_d = nc.dram_tensor("wh2", [HID, HID], F16, kind="ExternalInput").ap()
    wi1_d = nc.dram_tensor("wi1", [EMB, HID], F16, kind="ExternalInput").ap()
    wi2_d = nc.dram_tensor("wi2", [HID, HID], F16, kind="ExternalInput").ap()
    b1_d = nc.dram_tensor("b1t", [1, HID], F16, kind="ExternalInput").ap()
    b2_d = nc.dram_tensor("b2t", [1, HID], F16, kind="ExternalInput").ap()
    wd_d = nc.dram_tensor("wdk", [P, M], F16, kind="ExternalInput").ap()
    bd_d = nc.dram_tensor("bdv", [RB], F32, kind="ExternalInput").ap()
    y_d = nc.dram_tensor("y", [RB], F32, kind="ExternalOutput").ap()

    AF = mybir.ActivationFunctionType
    use_bias = bias1 or bias2

    with tile.TileContext(nc) as tc, ExitStack() as ctx:
        cpool = ctx.enter_context(tc.tile_pool(name="const", bufs=1))
        wpool = ctx.enter_context(tc.tile_pool(name="weights", bufs=1))
        gpool = ctx.enter_context(tc.tile_pool(name="gather", bufs=3))
        slpool = ctx.enter_context(tc.tile_pool(name="slab", bufs=2))
        a1pool = ctx.enter_context(tc.tile_pool(name="arch1", bufs=4))
        a2pool = ctx.enter_context(tc.tile_pool(name="arch2", bufs=4))
        p1pool = ctx.enter_context(tc.tile_pool(name="ps1", bufs=3, space="PSUM"))
        p2pool = ctx.enter_context(tc.tile_pool(name="ps2", bufs=3, space="PSUM"))
        tppool = ctx.enter_context(tc.tile_pool(name="tp", bufs=1, space="PSUM"))
        hpool = ctx.enter_context(tc.tile_pool(name="hp", bufs=1, space="PSUM"))

        ident = cpool.tile([P, P], F16, name="ident")
        make_identity(nc, ident[:])
        xg_sb = cpool.tile([W, NCH], I32, name="xg_sb")
        nc.sync.dma_start(out=xg_sb[:], in_=xg_d[:])
        wd_sb = cpool.tile([P, M], F16, name="wd_sb")
        nc.sync.dma_start(out=wd_sb[:], in_=wd_d[:])
        bd_sb = cpool.tile([P, 1], F32, name="bd_sb")
        nc.sync.dma_start(out=bd_sb[0:RB, 0:1], in_=bd_d[:])
        if use_bias:
            ones = cpool.tile([P, W], F16, name="ones")
            nc.vector.memset(ones[:], 0.0)
            nc.vector.memset(ones[0:1, :], 1.0)
            bt_sb = cpool.tile([P, 2 * HID], F16, name="bt_sb")
            nc.vector.memset(bt_sb[:], 0.0)
            nc.sync.dma_start(out=bt_sb[0:1, 0:HID], in_=b1_d[:])
            nc.sync.dma_start(out=bt_sb[0:1, HID:2 * HID], in_=b2_d[:])

        whs = {}
        for lyr, src in ((1, wh1_d), (2, wh2_d)):
            wsb = wpool.tile([P, KH * HID], F16, name=f"wh{lyr}_sb")
            for k in range(KH):
                nc.sync.dma_start(out=wsb[:, k * HID:(k + 1) * HID],
                                  in_=src[k * P:(k + 1) * P, :])
            whs[lyr] = wsb
        wi1_sb = wpool.tile([P, KE * HID], F16, name="wi1_sb")
        for k in range(KE):
            nc.sync.dma_start(out=wi1_sb[:, k * HID:(k + 1) * HID],
                              in_=wi1_d[k * P:(k + 1) * P, :])
        wi2_sb = wpool.tile([P, KH * HID], F16, name="wi2_sb")
        for k in range(KH):
            nc.sync.dma_start(out=wi2_sb[:, k * HID:(k + 1) * HID],
                              in_=wi2_d[k * P:(k + 1) * P, :])

        xes, slabs, ps1, ps2 = {}, {}, {}, {}
        arch1, arch2 = {}, {}

        def emit_gather(c):
            xe = gpool.tile([W, EMB], F16, tag="xe", name=f"xe{c}")
            nc.gpsimd.indirect_dma_start(
                out=xe[:], out_offset=None, in_=emb_d[:],
                in_offset=IndirectOffsetOnAxis(
                    ap=xg_sb[:, c:c + 1], axis=0))
            xes[c] = xe

        def slab_items(c):
            slab = slpool.tile([P, KE * W], F16, tag="slab", name=f"slab{c}")
            slabs[c] = slab
            items = []
            for e in range(KE):
                def tr(e=e, c=c, slab=slab):
                    pt = tppool.tile([P, W], F16, tag="tp", name=f"tp{c}_{e}")
                    nc.tensor.transpose(out=pt[:],
                                        in_=xes[c][:, e * P:(e + 1) * P],
                                        identity=ident[0:W, 0:W])
                    nc.vector.tensor_copy(out=slab[:, e * W:(e + 1) * W],
                                          in_=pt[:])
                items.append(tr)
            return items

        def prod_items(lyr, c):
            """Production matmuls writing pre-activations for chunk c of
            layer `lyr` directly into a fresh PSUM chunk bank."""
            pool, store = (p1pool, ps1) if lyr == 1 else (p2pool, ps2)
            ps = pool.tile([P, CW], F32, tag=f"ps{lyr}", name=f"ps{lyr}_{c}")
            store[c] = ps
            wsb, nk = (wi1_sb, KE) if lyr == 1 else (wi2_sb, KH)
            has_bias = bias1 if lyr == 1 else bias2
            items = []
            for m in range(M):
                for k in range(nk):
                    def mm(m=m, k=k, c=c, lyr=lyr, ps=ps, wsb=wsb):
                        if lyr == 1:
                            rhs = slabs[c][:, k * MB:(k + 1) * MB]
                        else:
                            rhs = arch1[c][:, k * MB:(k + 1) * MB]
                        nc.tensor.matmul(
                            ps[:, m * MB:(m + 1) * MB],
                            lhsT=wsb[:, (k * M + m) * P:(k * M + m + 1) * P],
                            rhs=rhs,
                            start=(m == 0 and k == 0), stop=False,
                            skip_group_check=True)
                    items.append(mm)
                if has_bias:
                    def bm(m=m, ps=ps, lyr=lyr):
                        off = (0 if lyr == 1 else HID) + m * P
                        nc.tensor.matmul(
                            ps[:, m * MB:(m + 1) * MB],
                            lhsT=bt_sb[:, off:off + P],
                            rhs=ones[:],
                            start=False, stop=False, skip_group_check=True)
                    items.append(bm)
            return items

        def scan_step(lyr, c, t):
            apool, store, psd = ((a1pool, arch1, ps1) if lyr == 1
                                 else (a2pool, arch2, ps2))
            wsb = whs[lyr]
            ps = psd[c]
            if t == 0:
                arch = apool.tile([P, CW], F16, tag=f"arch{lyr}",
                                  name=f"arch{lyr}_{c}")
                store[c] = arch
            else:
                arch = store[c]
            first = (c == 0 and t == 0)
            if not first:
                if t > 0:
                    rsrc, rt = arch, t - 1
                else:
                    rsrc, rt = store[c - 1], CS - 1
                for k in range(KH):
                    for m in range(M):
                        nc.tensor.matmul(
                            ps[:, m * MB + t * RB:m * MB + (t + 1) * RB],
                            lhsT=wsb[:, (k * M + m) * P:(k * M + m + 1) * P],
                            rhs=rsrc[:, k * MB + rt * RB:k * MB + (rt + 1) * RB],
                            start=False,
                            stop=(k == KH - 1 and m == M - 1),
                            skip_group_check=True)
            nc.scalar.activation(
                out=_mv(arch[:])[:, :, t * RB:(t + 1) * RB],
                in_=_mv(ps[:])[:, :, t * RB:(t + 1) * RB], func=AF.Tanh)

        # ---- prologue: chunk 0 slab + L1 production ----
        emit_gather(0)
        emit_gather(1)
        for it in slab_items(0):
            it()
        for it in prod_items(1, 0):
            it()

        # ---- main pipelined windows ----
        for w in range(NCH + LAG if nwin is None else nwin):
            items = []
            if w + 2 <= NCH - 1:
                items.append(lambda c=w + 2: emit_gather(c))
            if w + 1 <= NCH - 1:
                items += slab_items(w + 1)
                items += prod_items(1, w + 1)
            if 0 <= w - 1 <= NCH - 1:
                items += prod_items(2, w - 1)
            budget = (len(items) + 2 * CS - 1) // (2 * CS)
            for t in range(CS):
                if w <= NCH - 1:
                    scan_step(1, w, t)
                    for _ in range(budget):
                        if items:
                            items.pop(0)()
                if 0 <= w - LAG <= NCH - 1:
                    scan_step(2, w - LAG, t)
                    for _ in range(budget):
                        if items:
                            items.pop(0)()
            for it in items:
                it()

        # ---- head: y = sigmoid(h2_last @ Wd + bd) ----
        hps = hpool.tile([RB, 1], F32, tag="hps", name="hps")
        last = arch2[max(k for k in arch2)] if arch2 else arch1[max(k for k in arch1)]
        for m in range(M):
            nc.tensor.matmul(
                hps[:], lhsT=last[:, m * MB + (CS - 1) * RB:
                                 m * MB + CS * RB],
                rhs=wd_sb[:, m:m + 1], start=(m == 0), stop=(m == M - 1))
        y_sb = cpool.tile([P, 1], F32, name="y_sb")
        nc.scalar.activation(out=y_sb[0:RB, 0:1], in_=hps[:],
                             func=AF.Sigmoid, bias=bd_sb[0:RB, 0:1])
        nc.sync.dma_start(out=y_d[:], in_=y_sb[0:RB, 0:1])

    prune_redundant_self_waits(nc)
    nc.compile()
    return nc


def _prep_maps(x, emb, Wi1, Wh1, b1, Wi2, Wh2, b2, Wd, bd):
    x = np.asarray(x, np.int64)
    shared = {
        "emb": np.ascontiguousarray(np.asarray(emb, NPF)),
        "wh1": np.ascontiguousarray(np.asarray(Wh1, NPF)),
        "wh2": np.ascontiguousarray(np.asarray(Wh2, NPF)),
        "wi1": np.ascontiguousarray(np.asarray(Wi1, NPF)),
        "wi2": np.ascontiguousarray(np.asarray(Wi2, NPF)),
        "b1t": np.ascontiguousarray(np.asarray(b1, NPF).reshape(1, HID)),
        "b2t": np.ascontiguousarray(np.asarray(b2, NPF).reshape(1, HID)),
        "wdk": np.ascontiguousarray(np.asarray(Wd, NPF).reshape(M, P).T),
        "bdv": np.ascontiguousarray(
            np.broadcast_to(np.asarray(bd, np.float32), (RB,))),
    }
    in_maps = []
    for c in range(NCORES):
        xs = x[c * RB:(c + 1) * RB, :]                          # [8, 512]
        tok = np.ascontiguousarray(xs.T).reshape(NCH, CS * RB)  # (t, b) order
        xg = np.ascontiguousarray(tok.T.astype(np.int32))  # [64, NCH]
        in_maps.append({**shared, "xg": xg})
    return in_maps


def kernel(x, emb, Wi1, Wh1, b1, Wi2, Wh2, b2, Wd, bd):
    bias1 = bool(np.any(np.asarray(b1)))
    bias2 = bool(np.any(np.asarray(b2)))
    key = (bias1, bias2)
    if key not in _BUILT:
        _BUILT[key] = build(bias1=bias1, bias2=bias2)
    nc = _BUILT[key]
    in_maps = _prep_maps(x, emb, Wi1, Wh1, b1, Wi2, Wh2, b2, Wd, bd)
    res = run_bass_kernel_spmd(nc, in_maps, list(range(NCORES)))
    kernel.last_result = res
    y = np.concatenate([np.asarray(res.results[c]["y"], np.float32)
                        for c in range(NCORES)])
    return y


# revision 4
# speedup vs baseline: 1.2613x; 1.0553x over previous
"""Two-layer Elman RNN (B=64, S=512, EMB=512, HID=1024) on 8 TRN2 NeuronCores.

Pure data-parallel layout: core c owns batch rows [8c, 8c+8) and runs BOTH
layers itself as two software-pipelined recurrence chains — the layer-1 chain
at chunk k and the layer-2 chain LAG=2 chunks behind — so no collectives and
no DRAM round-trip for h1 are needed; the L2 chain's scan input is produced
locally from the L1 arch in SBUF.

Per chain-step the whole h update is ONE PSUM accumulation group in a
chunk-sized PSUM bank: the production pass (x@Wi1 for L1, h1@Wi2 for L2,
emitted a window ahead) writes the pre-activations for all CS=8 steps of the
chunk straight into the bank (first matmul start=True zero-arms the bank;
later first-touch writes auto-zero, subsequent writes accumulate), the 64
per-step Wh tile matmuls accumulate on top, and a single [128, 64] Tanh
drains the step's columns into the fp16 h archive. The critical chain per
step is tanh -> 64 matmuls -> tanh with one cross-engine semaphore hop each
direction; prune_redundant_self_waits removes tile's redundant self-engine
tick waits so dependent instructions carry their cross-engine wait directly
instead of behind a blocking EventSemaphore. The two chains' steps
interleave, filling each chain's latency window with the other chain's work
plus production/gather/transpose items.
"""

import collections
import re
from contextlib import ExitStack

import numpy as np

import concourse.bass as bass
import concourse.bacc as bacc
import concourse.mybir as mybir
import concourse.tile as tile
from concourse.bass import IndirectOffsetOnAxis
from concourse.bass_utils import run_bass_kernel_spmd
from concourse.masks import make_identity

P = 128
VOCAB, EMB, HID = 50257, 512, 1024
B, S = 64, 512
NCORES = 8
RB = B // NCORES          # 8 batch rows per core
M = HID // P              # 8 output feature blocks
KH = HID // P             # 8 contraction tiles (hidden)
KE = EMB // P             # 4 contraction tiles (embedding)
CS = 8                    # scan steps per chunk
NCH = S // CS             # 64 chunks
LAG = 2                   # chunks the L2 chain trails the L1 chain
W = M * RB                # 64 (m, b) columns per step
CW = CS * W               # 512 columns per chunk

F16 = mybir.dt.float16
F32 = mybir.dt.float32
I32 = mybir.dt.int32
NPF = np.float16

_BUILT = {}

_TICK = re.compile(r"^(PE|Activation|DVE|Pool|SP)_\d+$")


def prune_redundant_self_waits(nc):
    """Drop waits provably satisfied by same-engine program order.

    Tile assigns every instruction a wait on its own engine's tick semaphore;
    on TRN2 each instruction may carry at most ONE wait, so any cross-engine
    dependency then gets split into a separate blocking EventSemaphore that
    serializes the sequencer behind the wait and adds a post-release decode
    to the critical chain.  A wait `S >= V` on engine E is redundant iff S is
    E's own tick sem (only E's instructions increment it, once each, in
    order) and >= V increments precede this instruction in the same block.
    DMA-completion sems (DMAHW*/SWDGE) never match the tick-sem pattern.
    """
    for fn in nc.m.functions:
        for bb in fn.blocks:
            inc_count = collections.Counter()
            for inst in bb.instructions:
                si = inst.sync_info
                eng = getattr(inst.engine, "value", str(inst.engine))
                if si is not None and si.on_wait:
                    keep = []
                    for wt in si.on_wait:
                        mt = _TICK.match(wt.ant_name or "")
                        if (
                            mt is not None
                            and mt.group(1) == eng
                            and wt.sync_type == "semaphore"
                            and wt.wait_mode == "sem-ge-imm"
                            and wt.wait_reg is None
                            and inc_count.get(wt.ant_name, 0) >= wt.wait_value
                        ):
                            continue
                        keep.append(wt)
                    if len(keep) != len(si.on_wait):
                        inst.sync_info = type(si)(
                            on_wait=keep, on_update=list(si.on_update)
                        )
                si = inst.sync_info
                if si is not None:
                    for u in si.on_update:
                        if (
                            u.sync_type == "semaphore"
                            and u.update_mode == "sem-inc"
                            and _TICK.match(u.ant_name or "")
                        ):
                            inc_count[u.ant_name] += u.update_value


MB = CS * RB              # 64 cols per m-block in the (m, t, b) chunk layout


def _mv(ap):
    """View a [P, CW] chunk AP as [P, m, tb] with tb = MB cols per m-block."""
    return ap.rearrange("p (m tb) -> p m tb", tb=MB)


def build(local_cc=False, bias1=False, bias2=False, nwin=None):
    del local_cc  # no collectives in this kernel; kept for test harness compat
    nc = bacc.Bacc("TRN2", target_bir_lowering=False, debug=False,
                   num_devices=NCORES)

    xg_d = nc.dram_tensor("xg", [W, NCH], I32, kind="ExternalInput").ap()
    emb_d = nc.dram_tensor("emb", [VOCAB, EMB], F16, kind="ExternalInput").ap()
    wh1_d = nc.dram_tensor("wh1", [HID, HID], F16, kind="ExternalInput").ap()
    wh2_d = nc.dram_tensor("wh2", [HID, HID], F16, kind="ExternalInput").ap()
    wi1_d = nc.dram_tensor("wi1", [EMB, HID], F16, kind="ExternalInput").ap()
    wi2_d = nc.dram_tensor("wi2", [HID, HID], F16, kind="ExternalInput").ap()
    b1_d = nc.dram_tensor("b1t", [1, HID], F16, kind="ExternalInput").ap()
    b2_d = nc.dram_tensor("b2t", [1, HID], F16, kind="ExternalInput").ap()
    wd_d = nc.dram_tensor("wdk", [P, M], F16, kind="ExternalInput").ap()
    bd_d = nc.dram_tensor("bdv", [RB], F32, kind="ExternalInput").ap()
    y_d = nc.dram_tensor("y", [RB], F32, kind="ExternalOutput").ap()

    AF = mybir.ActivationFunctionType
    use_bias = bias1 or bias2

    with tile.TileContext(nc) as tc, ExitStack() as ctx:
        cpool = ctx.enter_context(tc.tile_pool(name="const", bufs=1))
        wpool = ctx.enter_context(tc.tile_pool(name="weights", bufs=1))
        gpool = ctx.enter_context(tc.tile_pool(name="gather", bufs=3))
        slpool = ctx.enter_context(tc.tile_pool(name="slab", bufs=2))
        a1pool = ctx.enter_context(tc.tile_pool(name="arch1", bufs=4))
        a2pool = ctx.enter_context(tc.tile_pool(name="arch2", bufs=4))
        p1pool = ctx.enter_context(tc.tile_pool(name="ps1", bufs=3, space="PSUM"))
        p2pool = ctx.enter_context(tc.tile_pool(name="ps2", bufs=3, space="PSUM"))
        tppool = ctx.enter_context(tc.tile_pool(name="tp", bufs=2, space="PSUM"))

        ident = cpool.tile([P, P], F16, name="ident")
        make_identity(nc, ident[:])
        xg_sb = cpool.tile([W, NCH], I32, name="xg_sb")
        nc.sync.dma_start(out=xg_sb[:], in_=xg_d[:])
        if use_bias:
            ones = cpool.tile([P, W], F16, name="ones")
            nc.vector.memset(ones[:], 0.0)
            nc.vector.memset(ones[0:1, :], 1.0)
            bt_sb = cpool.tile([P, 2 * HID], F16, name="bt_sb")
            nc.vector.memset(bt_sb[:], 0.0)
            nc.sync.dma_start(out=bt_sb[0:1, 0:HID], in_=b1_d[:])
            nc.sync.dma_start(out=bt_sb[0:1, HID:2 * HID], in_=b2_d[:])

        xes, slabs, ps1, ps2 = {}, {}, {}, {}
        arch1, arch2 = {}, {}

        def emit_gather(c):
            xe = gpool.tile([W, EMB], F16, tag="xe", name=f"xe{c}")
            nc.gpsimd.indirect_dma_start(
                out=xe[:], out_offset=None, in_=emb_d[:],
                in_offset=IndirectOffsetOnAxis(
                    ap=xg_sb[:, c:c + 1], axis=0))
            xes[c] = xe

        def slab_items(c):
            slab = slpool.tile([P, KE * W], F16, tag="slab", name=f"slab{c}")
            slabs[c] = slab
            items = []
            for e in range(KE):
                def tr(e=e, c=c, slab=slab):
                    pt = tppool.tile([P, W], F16, tag="tp", name=f"tp{c}_{e}")
                    nc.tensor.transpose(out=pt[:],
                                        in_=xes[c][:, e * P:(e + 1) * P],
                                        identity=ident[0:W, 0:W])
                    nc.vector.tensor_copy(out=slab[:, e * W:(e + 1) * W],
                                          in_=pt[:])
                items.append(tr)
            return items

        def prod_items(lyr, c):
            """Production matmuls writing pre-activations for chunk c of
            layer `lyr` directly into a fresh PSUM chunk bank."""
            pool, store = (p1pool, ps1) if lyr == 1 else (p2pool, ps2)
            ps = pool.tile([P, CW], F32, tag=f"ps{lyr}", name=f"ps{lyr}_{c}")
            store[c] = ps
            wsb, nk = (wi1_sb, KE) if lyr == 1 else (wi2_sb, KH)
            has_bias = bias1 if lyr == 1 else bias2
            items = []
            for m in range(M):
                for k in range(nk):
                    def mm(m=m, k=k, c=c, lyr=lyr, ps=ps, wsb=wsb):
                        if lyr == 1:
                            rhs = slabs[c][:, k * MB:(k + 1) * MB]
                        else:
                            rhs = arch1[c][:, k * MB:(k + 1) * MB]
                        nc.tensor.matmul(
                            ps[:, m * MB:(m + 1) * MB],
                            lhsT=wsb[:, (k * M + m) * P:(k * M + m + 1) * P],
                            rhs=rhs,
                            start=(m == 0 and k == 0), stop=False,
                            skip_group_check=True)
                    items.append(mm)
                if has_bias:
                    def bm(m=m, ps=ps, lyr=lyr):
                        off = (0 if lyr == 1 else HID) + m * P
                        nc.tensor.matmul(
                            ps[:, m * MB:(m + 1) * MB],
                            lhsT=bt_sb[:, off:off + P],
                            rhs=ones[:],
                            start=False, stop=False, skip_group_check=True)
                    items.append(bm)
            return items

        def scan_step(lyr, c, t):
            apool, store, psd = ((a1pool, arch1, ps1) if lyr == 1
                                 else (a2pool, arch2, ps2))
            wsb = whs[lyr]
            ps = psd[c]
            if t == 0:
                arch = apool.tile([P, CW], F16, tag=f"arch{lyr}",
                                  name=f"arch{lyr}_{c}")
                store[c] = arch
            else:
                arch = store[c]
            first = (c == 0 and t == 0)
            if not first:
                if t > 0:
                    rsrc, rt = arch, t - 1
                else:
                    rsrc, rt = store[c - 1], CS - 1
                for k in range(KH):
                    for m in range(M):
                        nc.tensor.matmul(
                            ps[:, m * MB + t * RB:m * MB + (t + 1) * RB],
                            lhsT=wsb[:, (k * M + m) * P:(k * M + m + 1) * P],
                            rhs=rsrc[:, k * MB + rt * RB:k * MB + (rt + 1) * RB],
                            start=False,
                            stop=(k == KH - 1 and m == M - 1),
                            skip_group_check=True)
            nc.scalar.activation(
                out=_mv(arch[:])[:, :, t * RB:(t + 1) * RB],
                in_=_mv(ps[:])[:, :, t * RB:(t + 1) * RB], func=AF.Tanh)

        # ---- prologue: gathers first, then weights, chunk-0 slab + prod ----
        emit_gather(0)
        emit_gather(1)
        wd_sb = cpool.tile([P, M], F16, name="wd_sb")
        nc.sync.dma_start(out=wd_sb[:], in_=wd_d[:])
        bd_sb = cpool.tile([P, 1], F32, name="bd_sb")
        nc.sync.dma_start(out=bd_sb[0:RB, 0:1], in_=bd_d[:])
        whs = {}
        wi1_sb = wpool.tile([P, KE * HID], F16, name="wi1_sb")
        for k in range(KE):
            nc.sync.dma_start(out=wi1_sb[:, k * HID:(k + 1) * HID],
                              in_=wi1_d[k * P:(k + 1) * P, :])
        wsb1 = wpool.tile([P, KH * HID], F16, name="wh1_sb")
        for k in range(KH):
            nc.sync.dma_start(out=wsb1[:, k * HID:(k + 1) * HID],
                              in_=wh1_d[k * P:(k + 1) * P, :])
        whs[1] = wsb1
        wi2_sb = wpool.tile([P, KH * HID], F16, name="wi2_sb")
        for k in range(KH):
            nc.sync.dma_start(out=wi2_sb[:, k * HID:(k + 1) * HID],
                              in_=wi2_d[k * P:(k + 1) * P, :])
        wsb2 = wpool.tile([P, KH * HID], F16, name="wh2_sb")
        for k in range(KH):
            nc.sync.dma_start(out=wsb2[:, k * HID:(k + 1) * HID],
                              in_=wh2_d[k * P:(k + 1) * P, :])
        whs[2] = wsb2
        for it in slab_items(0):
            it()
        for it in prod_items(1, 0):
            it()

        # ---- main pipelined windows ----
        for w in range(NCH + LAG if nwin is None else nwin):
            items = []
            if w + 2 <= NCH - 1:
                items.append(lambda c=w + 2: emit_gather(c))
            if w + 1 <= NCH - 1:
                items += slab_items(w + 1)
                items += prod_items(1, w + 1)
            if 0 <= w - 1 <= NCH - 1:
                items += prod_items(2, w - 1)
            budget = (len(items) + 9) // 10
            for t in range(CS):
                if w <= NCH - 1:
                    scan_step(1, w, t)
                    for _ in range(budget):
                        if items:
                            items.pop(0)()
                if 0 <= w - LAG <= NCH - 1:
                    scan_step(2, w - LAG, t)
                    for _ in range(budget):
                        if items:
                            items.pop(0)()
            for it in items:
                it()

        # ---- head: y = sigmoid(h2_last @ Wd + bd) ----
        hps = p1pool.tile([RB, 1], F32, tag="ps1", name="hps")
        last = arch2[max(k for k in arch2)] if arch2 else arch1[max(k for k in arch1)]
        for m in range(M):
            nc.tensor.matmul(
                hps[:], lhsT=last[:, m * MB + (CS - 1) * RB:
                                 m * MB + CS * RB],
                rhs=wd_sb[:, m:m + 1], start=(m == 0), stop=(m == M - 1))
        y_sb = cpool.tile([P, 1], F32, name="y_sb")
        nc.scalar.activation(out=y_sb[0:RB, 0:1], in_=hps[:],
                             func=AF.Sigmoid, bias=bd_sb[0:RB, 0:1])
        nc.sync.dma_start(out=y_d[:], in_=y_sb[0:RB, 0:1])

    prune_redundant_self_waits(nc)
    nc.compile()
    return nc


def _prep_maps(x, emb, Wi1, Wh1, b1, Wi2, Wh2, b2, Wd, bd):
    x = np.asarray(x, np.int64)
    shared = {
        "emb": np.ascontiguousarray(np.asarray(emb, NPF)),
        "wh1": np.ascontiguousarray(np.asarray(Wh1, NPF)),
        "wh2": np.ascontiguousarray(np.asarray(Wh2, NPF)),
        "wi1": np.ascontiguousarray(np.asarray(Wi1, NPF)),
        "wi2": np.ascontiguousarray(np.asarray(Wi2, NPF)),
        "b1t": np.ascontiguousarray(np.asarray(b1, NPF).reshape(1, HID)),
        "b2t": np.ascontiguousarray(np.asarray(b2, NPF).reshape(1, HID)),
        "wdk": np.ascontiguousarray(np.asarray(Wd, NPF).reshape(M, P).T),
        "bdv": np.ascontiguousarray(
            np.broadcast_to(np.asarray(bd, np.float32), (RB,))),
    }
    in_maps = []
    for c in range(NCORES):
        xs = x[c * RB:(c + 1) * RB, :]                          # [8, 512]
        tok = np.ascontiguousarray(xs.T).reshape(NCH, CS * RB)  # (t, b) order
        xg = np.ascontiguousarray(tok.T.astype(np.int32))  # [64, NCH]
        in_maps.append({**shared, "xg": xg})
    return in_maps


def kernel(x, emb, Wi1, Wh1, b1, Wi2, Wh2, b2, Wd, bd):
    bias1 = bool(np.any(np.asarray(b1)))
    bias2 = bool(np.any(np.asarray(b2)))
    key = (bias1, bias2)
    if key not in _BUILT:
        _BUILT[key] = build(bias1=bias1, bias2=bias2)
    nc = _BUILT[key]
    in_maps = _prep_maps(x, emb, Wi1, Wh1, b1, Wi2, Wh2, b2, Wd, bd)
    res = run_bass_kernel_spmd(nc, in_maps, list(range(NCORES)))
    kernel.last_result = res
    y = np.concatenate([np.asarray(res.results[c]["y"], np.float32)
                        for c in range(NCORES)])
    return y


# revision 5
# speedup vs baseline: 1.2640x; 1.0022x over previous
"""Two-layer Elman RNN (B=64, S=512, EMB=512, HID=1024) on 8 TRN2 NeuronCores.

Pure data-parallel layout: core c owns batch rows [8c, 8c+8) and runs BOTH
layers itself as two software-pipelined recurrence chains — the layer-1 chain
at chunk k and the layer-2 chain LAG=2 chunks behind — so no collectives and
no DRAM round-trip for h1 are needed; the L2 chain's scan input is produced
locally from the L1 arch in SBUF.

Per chain-step the whole h update is ONE PSUM accumulation group in a
chunk-sized PSUM bank: the production pass (x@Wi1 for L1, h1@Wi2 for L2,
emitted a window ahead) writes the pre-activations for all CS=8 steps of the
chunk straight into the bank (first matmul start=True zero-arms the bank;
later first-touch writes auto-zero, subsequent writes accumulate), the 64
per-step Wh tile matmuls accumulate on top, and a single [128, 64] Tanh
drains the step's columns into the fp16 h archive. The critical chain per
step is tanh -> 64 matmuls -> tanh with one cross-engine semaphore hop each
direction; prune_redundant_self_waits removes tile's redundant self-engine
tick waits so dependent instructions carry their cross-engine wait directly
instead of behind a blocking EventSemaphore. The two chains' steps
interleave, filling each chain's latency window with the other chain's work
plus production/gather/transpose items.
"""

import collections
import re
from contextlib import ExitStack

import numpy as np

import concourse.bass as bass
import concourse.bacc as bacc
import concourse.mybir as mybir
import concourse.tile as tile
from concourse.bass import IndirectOffsetOnAxis
from concourse.bass_utils import run_bass_kernel_spmd
from concourse.masks import make_identity

P = 128
VOCAB, EMB, HID = 50257, 512, 1024
B, S = 64, 512
NCORES = 8
RB = B // NCORES          # 8 batch rows per core
M = HID // P              # 8 output feature blocks
KH = HID // P             # 8 contraction tiles (hidden)
KE = EMB // P             # 4 contraction tiles (embedding)
CS = 8                    # scan steps per chunk
NCH = S // CS             # 64 chunks
LAG = 2                   # chunks the L2 chain trails the L1 chain
W = M * RB                # 64 (m, b) columns per step
CW = CS * W               # 512 columns per chunk

F16 = mybir.dt.float16
F32 = mybir.dt.float32
I32 = mybir.dt.int32
NPF = np.float16

_BUILT = {}

_TICK = re.compile(r"^(PE|Activation|DVE|Pool|SP)_\d+$")


def prune_redundant_self_waits(nc):
    """Drop waits provably satisfied by same-engine program order.

    Tile assigns every instruction a wait on its own engine's tick semaphore;
    on TRN2 each instruction may carry at most ONE wait, so any cross-engine
    dependency then gets split into a separate blocking EventSemaphore that
    serializes the sequencer behind the wait and adds a post-release decode
    to the critical chain.  A wait `S >= V` on engine E is redundant iff S is
    E's own tick sem (only E's instructions increment it, once each, in
    order) and >= V increments precede this instruction in the same block.
    DMA-completion sems (DMAHW*/SWDGE) never match the tick-sem pattern.
    """
    for fn in nc.m.functions:
        for bb in fn.blocks:
            inc_count = collections.Counter()
            for inst in bb.instructions:
                si = inst.sync_info
                eng = getattr(inst.engine, "value", str(inst.engine))
                if si is not None and si.on_wait:
                    keep = []
                    for wt in si.on_wait:
                        mt = _TICK.match(wt.ant_name or "")
                        if (
                            mt is not None
                            and mt.group(1) == eng
                            and wt.sync_type == "semaphore"
                            and wt.wait_mode == "sem-ge-imm"
                            and wt.wait_reg is None
                            and inc_count.get(wt.ant_name, 0) >= wt.wait_value
                        ):
                            continue
                        keep.append(wt)
                    if len(keep) != len(si.on_wait):
                        inst.sync_info = type(si)(
                            on_wait=keep, on_update=list(si.on_update)
                        )
                si = inst.sync_info
                if si is not None:
                    for u in si.on_update:
                        if (
                            u.sync_type == "semaphore"
                            and u.update_mode == "sem-inc"
                            and _TICK.match(u.ant_name or "")
                        ):
                            inc_count[u.ant_name] += u.update_value


MB = CS * RB              # 64 cols per m-block in the (m, t, b) chunk layout


def _mv(ap):
    """View a [P, CW] chunk AP as [P, m, tb] with tb = MB cols per m-block."""
    return ap.rearrange("p (m tb) -> p m tb", tb=MB)


def build(local_cc=False, bias1=False, bias2=False, nwin=None):
    del local_cc  # no collectives in this kernel; kept for test harness compat
    nc = bacc.Bacc("TRN2", target_bir_lowering=False, debug=False,
                   num_devices=NCORES)

    xg_d = nc.dram_tensor("xg", [W, NCH], I32, kind="ExternalInput").ap()
    emb_d = nc.dram_tensor("emb", [VOCAB, EMB], F16, kind="ExternalInput").ap()
    wh1_d = nc.dram_tensor("wh1", [HID, HID], F16, kind="ExternalInput").ap()
    wh2_d = nc.dram_tensor("wh2", [HID, HID], F16, kind="ExternalInput").ap()
    wi1_d = nc.dram_tensor("wi1", [EMB, HID], F16, kind="ExternalInput").ap()
    wi2_d = nc.dram_tensor("wi2", [HID, HID], F16, kind="ExternalInput").ap()
    b1_d = nc.dram_tensor("b1t", [1, HID], F16, kind="ExternalInput").ap()
    b2_d = nc.dram_tensor("b2t", [1, HID], F16, kind="ExternalInput").ap()
    wd_d = nc.dram_tensor("wdk", [P, M], F16, kind="ExternalInput").ap()
    bd_d = nc.dram_tensor("bdv", [RB], F32, kind="ExternalInput").ap()
    y_d = nc.dram_tensor("y", [RB], F32, kind="ExternalOutput").ap()

    AF = mybir.ActivationFunctionType
    use_bias = bias1 or bias2

    with tile.TileContext(nc) as tc, ExitStack() as ctx:
        cpool = ctx.enter_context(tc.tile_pool(name="const", bufs=1))
        wpool = ctx.enter_context(tc.tile_pool(name="weights", bufs=1))
        gpool = ctx.enter_context(tc.tile_pool(name="gather", bufs=3))
        slpool = ctx.enter_context(tc.tile_pool(name="slab", bufs=2))
        a1pool = ctx.enter_context(tc.tile_pool(name="arch1", bufs=4))
        a2pool = ctx.enter_context(tc.tile_pool(name="arch2", bufs=4))
        p1pool = ctx.enter_context(tc.tile_pool(name="ps1", bufs=3, space="PSUM"))
        p2pool = ctx.enter_context(tc.tile_pool(name="ps2", bufs=3, space="PSUM"))
        tppool = ctx.enter_context(tc.tile_pool(name="tp", bufs=2, space="PSUM"))

        ident = cpool.tile([P, P], F16, name="ident")
        make_identity(nc, ident[:])
        xg_sb = cpool.tile([W, NCH], I32, name="xg_sb")
        nc.sync.dma_start(out=xg_sb[:], in_=xg_d[:])
        if use_bias:
            ones = cpool.tile([P, W], F16, name="ones")
            nc.vector.memset(ones[:], 0.0)
            nc.vector.memset(ones[0:1, :], 1.0)
            bt_sb = cpool.tile([P, 2 * HID], F16, name="bt_sb")
            nc.vector.memset(bt_sb[:], 0.0)
            nc.sync.dma_start(out=bt_sb[0:1, 0:HID], in_=b1_d[:])
            nc.sync.dma_start(out=bt_sb[0:1, HID:2 * HID], in_=b2_d[:])

        xes, slabs, ps1, ps2 = {}, {}, {}, {}
        arch1, arch2 = {}, {}

        def emit_gather(c):
            xe = gpool.tile([W, EMB], F16, tag="xe", name=f"xe{c}")
            nc.gpsimd.indirect_dma_start(
                out=xe[:], out_offset=None, in_=emb_d[:],
                in_offset=IndirectOffsetOnAxis(
                    ap=xg_sb[:, c:c + 1], axis=0))
            xes[c] = xe

        def slab_items(c):
            slab = slpool.tile([P, KE * W], F16, tag="slab", name=f"slab{c}")
            slabs[c] = slab
            items = []
            for e in range(KE):
                def tr(e=e, c=c, slab=slab):
                    pt = tppool.tile([P, W], F16, tag="tp", name=f"tp{c}_{e}")
                    nc.tensor.transpose(out=pt[:],
                                        in_=xes[c][:, e * P:(e + 1) * P],
                                        identity=ident[0:W, 0:W])
                    nc.vector.tensor_copy(out=slab[:, e * W:(e + 1) * W],
                                          in_=pt[:])
                items.append(tr)
            return items

        def prod_items(lyr, c):
            """Production matmuls writing pre-activations for chunk c of
            layer `lyr` directly into a fresh PSUM chunk bank."""
            pool, store = (p1pool, ps1) if lyr == 1 else (p2pool, ps2)
            ps = pool.tile([P, CW], F32, tag=f"ps{lyr}", name=f"ps{lyr}_{c}")
            store[c] = ps
            wsb, nk = (wi1_sb, KE) if lyr == 1 else (wi2_sb, KH)
            has_bias = bias1 if lyr == 1 else bias2
            items = []
            for m in range(M):
                for k in range(nk):
                    def mm(m=m, k=k, c=c, lyr=lyr, ps=ps, wsb=wsb):
                        if lyr == 1:
                            rhs = slabs[c][:, k * MB:(k + 1) * MB]
                        else:
                            rhs = arch1[c][:, k * MB:(k + 1) * MB]
                        nc.tensor.matmul(
                            ps[:, m * MB:(m + 1) * MB],
                            lhsT=wsb[:, (k * M + m) * P:(k * M + m + 1) * P],
                            rhs=rhs,
                            start=(m == 0 and k == 0), stop=False,
                            skip_group_check=True)
                    items.append(mm)
                if has_bias:
                    def bm(m=m, ps=ps, lyr=lyr):
                        off = (0 if lyr == 1 else HID) + m * P
                        nc.tensor.matmul(
                            ps[:, m * MB:(m + 1) * MB],
                            lhsT=bt_sb[:, off:off + P],
                            rhs=ones[:],
                            start=False, stop=False, skip_group_check=True)
                    items.append(bm)
            return items

        def scan_step(lyr, c, t):
            apool, store, psd = ((a1pool, arch1, ps1) if lyr == 1
                                 else (a2pool, arch2, ps2))
            wsb = whs[lyr]
            ps = psd[c]
            if t == 0:
                arch = apool.tile([P, CW], F16, tag=f"arch{lyr}",
                                  name=f"arch{lyr}_{c}")
                store[c] = arch
            else:
                arch = store[c]
            first = (c == 0 and t == 0)
            if not first:
                if t > 0:
                    rsrc, rt = arch, t - 1
                else:
                    rsrc, rt = store[c - 1], CS - 1
                for k in range(KH):
                    for m in range(M):
                        nc.tensor.matmul(
                            ps[:, m * MB + t * RB:m * MB + (t + 1) * RB],
                            lhsT=wsb[:, (k * M + m) * P:(k * M + m + 1) * P],
                            rhs=rsrc[:, k * MB + rt * RB:k * MB + (rt + 1) * RB],
                            start=False,
                            stop=(k == KH - 1 and m == M - 1),
                            skip_group_check=True)
            nc.scalar.activation(
                out=_mv(arch[:])[:, :, t * RB:(t + 1) * RB],
                in_=_mv(ps[:])[:, :, t * RB:(t + 1) * RB], func=AF.Tanh)

        # ---- prologue: gathers first, then weights, chunk-0 slab + prod ----
        emit_gather(0)
        emit_gather(1)
        wd_sb = cpool.tile([P, M], F16, name="wd_sb")
        nc.sync.dma_start(out=wd_sb[:], in_=wd_d[:])
        bd_sb = cpool.tile([P, 1], F32, name="bd_sb")
        nc.sync.dma_start(out=bd_sb[0:RB, 0:1], in_=bd_d[:])
        whs = {}
        wsb1 = wpool.tile([P, KH * HID], F16, name="wh1_sb")
        for k in range(KH):
            nc.sync.dma_start(out=wsb1[:, k * HID:(k + 1) * HID],
                              in_=wh1_d[k * P:(k + 1) * P, :])
        whs[1] = wsb1
        wi1_sb = wpool.tile([P, KE * HID], F16, name="wi1_sb")
        for k in range(KE):
            nc.sync.dma_start(out=wi1_sb[:, k * HID:(k + 1) * HID],
                              in_=wi1_d[k * P:(k + 1) * P, :])
        wi2_sb = wpool.tile([P, KH * HID], F16, name="wi2_sb")
        for k in range(KH):
            nc.sync.dma_start(out=wi2_sb[:, k * HID:(k + 1) * HID],
                              in_=wi2_d[k * P:(k + 1) * P, :])
        wsb2 = wpool.tile([P, KH * HID], F16, name="wh2_sb")
        for k in range(KH):
            nc.sync.dma_start(out=wsb2[:, k * HID:(k + 1) * HID],
                              in_=wh2_d[k * P:(k + 1) * P, :])
        whs[2] = wsb2
        for it in slab_items(0):
            it()
        for it in prod_items(1, 0):
            it()

        # ---- main pipelined windows ----
        for w in range(NCH + LAG if nwin is None else nwin):
            items = []
            if w + 2 <= NCH - 1:
                items.append(lambda c=w + 2: emit_gather(c))
            if w + 1 <= NCH - 1:
                items += slab_items(w + 1)
                items += prod_items(1, w + 1)
            if 0 <= w - 1 <= NCH - 1:
                items += prod_items(2, w - 1)
            budget = (len(items) + 9) // 10
            for t in range(CS):
                if w <= NCH - 1:
                    scan_step(1, w, t)
                    for _ in range(budget):
                        if items:
                            items.pop(0)()
                if 0 <= w - LAG <= NCH - 1:
                    scan_step(2, w - LAG, t)
                    for _ in range(budget):
                        if items:
                            items.pop(0)()
            for it in items:
                it()

        # ---- head: y = sigmoid(h2_last @ Wd + bd) ----
        hps = p1pool.tile([RB, 1], F32, tag="ps1", name="hps")
        last = arch2[max(k for k in arch2)] if arch2 else arch1[max(k for k in arch1)]
        for m in range(M):
            nc.tensor.matmul(
                hps[:], lhsT=last[:, m * MB + (CS - 1) * RB:
                                 m * MB + CS * RB],
                rhs=wd_sb[:, m:m + 1], start=(m == 0), stop=(m == M - 1))
        y_sb = cpool.tile([P, 1], F32, name="y_sb")
        nc.scalar.activation(out=y_sb[0:RB, 0:1], in_=hps[:],
                             func=AF.Sigmoid, bias=bd_sb[0:RB, 0:1])
        nc.sync.dma_start(out=y_d[:], in_=y_sb[0:RB, 0:1])

    prune_redundant_self_waits(nc)
    nc.compile()
    return nc


def _prep_maps(x, emb, Wi1, Wh1, b1, Wi2, Wh2, b2, Wd, bd):
    x = np.asarray(x, np.int64)
    shared = {
        "emb": np.ascontiguousarray(np.asarray(emb, NPF)),
        "wh1": np.ascontiguousarray(np.asarray(Wh1, NPF)),
        "wh2": np.ascontiguousarray(np.asarray(Wh2, NPF)),
        "wi1": np.ascontiguousarray(np.asarray(Wi1, NPF)),
        "wi2": np.ascontiguousarray(np.asarray(Wi2, NPF)),
        "b1t": np.ascontiguousarray(np.asarray(b1, NPF).reshape(1, HID)),
        "b2t": np.ascontiguousarray(np.asarray(b2, NPF).reshape(1, HID)),
        "wdk": np.ascontiguousarray(np.asarray(Wd, NPF).reshape(M, P).T),
        "bdv": np.ascontiguousarray(
            np.broadcast_to(np.asarray(bd, np.float32), (RB,))),
    }
    in_maps = []
    for c in range(NCORES):
        xs = x[c * RB:(c + 1) * RB, :]                          # [8, 512]
        tok = np.ascontiguousarray(xs.T).reshape(NCH, CS * RB)  # (t, b) order
        xg = np.ascontiguousarray(tok.T.astype(np.int32))  # [64, NCH]
        in_maps.append({**shared, "xg": xg})
    return in_maps


def kernel(x, emb, Wi1, Wh1, b1, Wi2, Wh2, b2, Wd, bd):
    bias1 = bool(np.any(np.asarray(b1)))
    bias2 = bool(np.any(np.asarray(b2)))
    key = (bias1, bias2)
    if key not in _BUILT:
        _BUILT[key] = build(bias1=bias1, bias2=bias2)
    nc = _BUILT[key]
    in_maps = _prep_maps(x, emb, Wi1, Wh1, b1, Wi2, Wh2, b2, Wd, bd)
    res = run_bass_kernel_spmd(nc, in_maps, list(range(NCORES)))
    kernel.last_result = res
    y = np.concatenate([np.asarray(res.results[c]["y"], np.float32)
                        for c in range(NCORES)])
    return y


# revision 6
# speedup vs baseline: 1.2659x; 1.0015x over previous
"""Two-layer Elman RNN (B=64, S=512, EMB=512, HID=1024) on 8 TRN2 NeuronCores.

Pure data-parallel layout: core c owns batch rows [8c, 8c+8) and runs BOTH
layers itself as two software-pipelined recurrence chains — the layer-1 chain
at chunk k and the layer-2 chain LAG=2 chunks behind — so no collectives and
no DRAM round-trip for h1 are needed; the L2 chain's scan input is produced
locally from the L1 arch in SBUF.

Per chain-step the whole h update is ONE PSUM accumulation group in a
chunk-sized PSUM bank: the production pass (x@Wi1 for L1, h1@Wi2 for L2,
emitted a window ahead) writes the pre-activations for all CS=8 steps of the
chunk straight into the bank (first matmul start=True zero-arms the bank;
later first-touch writes auto-zero, subsequent writes accumulate), the 64
per-step Wh tile matmuls accumulate on top, and a single [128, 64] Tanh
drains the step's columns into the fp16 h archive. The critical chain per
step is tanh -> 64 matmuls -> tanh with one cross-engine semaphore hop each
direction; prune_redundant_self_waits removes tile's redundant self-engine
tick waits so dependent instructions carry their cross-engine wait directly
instead of behind a blocking EventSemaphore. The two chains' steps
interleave, filling each chain's latency window with the other chain's work
plus production/gather/transpose items.
"""

import collections
import re
from contextlib import ExitStack

import numpy as np

import concourse.bass as bass
import concourse.bacc as bacc
import concourse.mybir as mybir
import concourse.tile as tile
from concourse.bass import IndirectOffsetOnAxis
from concourse.bass_utils import run_bass_kernel_spmd
from concourse.masks import make_identity

P = 128
VOCAB, EMB, HID = 50257, 512, 1024
B, S = 64, 512
NCORES = 8
RB = B // NCORES          # 8 batch rows per core
M = HID // P              # 8 output feature blocks
KH = HID // P             # 8 contraction tiles (hidden)
KE = EMB // P             # 4 contraction tiles (embedding)
CS = 8                    # scan steps per chunk
NCH = S // CS             # 64 chunks
LAG = 2                   # chunks the L2 chain trails the L1 chain
W = M * RB                # 64 (m, b) columns per step
CW = CS * W               # 512 columns per chunk

F16 = mybir.dt.float16
F32 = mybir.dt.float32
I32 = mybir.dt.int32
NPF = np.float16

_BUILT = {}

_TICK = re.compile(r"^(PE|Activation|DVE|Pool|SP)_\d+$")


def prune_redundant_self_waits(nc):
    """Drop waits provably satisfied by same-engine program order.

    Tile assigns every instruction a wait on its own engine's tick semaphore;
    on TRN2 each instruction may carry at most ONE wait, so any cross-engine
    dependency then gets split into a separate blocking EventSemaphore that
    serializes the sequencer behind the wait and adds a post-release decode
    to the critical chain.  A wait `S >= V` on engine E is redundant iff S is
    E's own tick sem (only E's instructions increment it, once each, in
    order) and >= V increments precede this instruction in the same block.
    DMA-completion sems (DMAHW*/SWDGE) never match the tick-sem pattern.
    """
    for fn in nc.m.functions:
        for bb in fn.blocks:
            inc_count = collections.Counter()
            for inst in bb.instructions:
                si = inst.sync_info
                eng = getattr(inst.engine, "value", str(inst.engine))
                if si is not None and si.on_wait:
                    keep = []
                    for wt in si.on_wait:
                        mt = _TICK.match(wt.ant_name or "")
                        if (
                            mt is not None
                            and mt.group(1) == eng
                            and wt.sync_type == "semaphore"
                            and wt.wait_mode == "sem-ge-imm"
                            and wt.wait_reg is None
                            and inc_count.get(wt.ant_name, 0) >= wt.wait_value
                        ):
                            continue
                        keep.append(wt)
                    if len(keep) != len(si.on_wait):
                        inst.sync_info = type(si)(
                            on_wait=keep, on_update=list(si.on_update)
                        )
                si = inst.sync_info
                if si is not None:
                    for u in si.on_update:
                        if (
                            u.sync_type == "semaphore"
                            and u.update_mode == "sem-inc"
                            and _TICK.match(u.ant_name or "")
                        ):
                            inc_count[u.ant_name] += u.update_value


MB = CS * RB              # 64 cols per m-block in the (m, t, b) chunk layout


def _mv(ap):
    """View a [P, CW] chunk AP as [P, m, tb] with tb = MB cols per m-block."""
    return ap.rearrange("p (m tb) -> p m tb", tb=MB)


def build(local_cc=False, bias1=False, bias2=False, nwin=None):
    del local_cc  # no collectives in this kernel; kept for test harness compat
    nc = bacc.Bacc("TRN2", target_bir_lowering=False, debug=False,
                   num_devices=NCORES)

    xg_d = nc.dram_tensor("xg", [W, NCH], I32, kind="ExternalInput").ap()
    emb_d = nc.dram_tensor("emb", [VOCAB, EMB], F16, kind="ExternalInput").ap()
    wh1_d = nc.dram_tensor("wh1", [HID, HID], F16, kind="ExternalInput").ap()
    wh2_d = nc.dram_tensor("wh2", [HID, HID], F16, kind="ExternalInput").ap()
    wi1_d = nc.dram_tensor("wi1", [EMB, HID], F16, kind="ExternalInput").ap()
    wi2_d = nc.dram_tensor("wi2", [HID, HID], F16, kind="ExternalInput").ap()
    b1_d = nc.dram_tensor("b1t", [1, HID], F16, kind="ExternalInput").ap()
    b2_d = nc.dram_tensor("b2t", [1, HID], F16, kind="ExternalInput").ap()
    wd_d = nc.dram_tensor("wdk", [P, M], F16, kind="ExternalInput").ap()
    bd_d = nc.dram_tensor("bdv", [RB], F32, kind="ExternalInput").ap()
    y_d = nc.dram_tensor("y", [RB], F32, kind="ExternalOutput").ap()

    AF = mybir.ActivationFunctionType
    use_bias = bias1 or bias2

    with tile.TileContext(nc) as tc, ExitStack() as ctx:
        cpool = ctx.enter_context(tc.tile_pool(name="const", bufs=1))
        wpool = ctx.enter_context(tc.tile_pool(name="weights", bufs=1))
        gpool = ctx.enter_context(tc.tile_pool(name="gather", bufs=3))
        slpool = ctx.enter_context(tc.tile_pool(name="slab", bufs=2))
        a1pool = ctx.enter_context(tc.tile_pool(name="arch1", bufs=4))
        a2pool = ctx.enter_context(tc.tile_pool(name="arch2", bufs=4))
        p1pool = ctx.enter_context(tc.tile_pool(name="ps1", bufs=3, space="PSUM"))
        p2pool = ctx.enter_context(tc.tile_pool(name="ps2", bufs=3, space="PSUM"))
        tppool = ctx.enter_context(tc.tile_pool(name="tp", bufs=2, space="PSUM"))

        ident = cpool.tile([P, P], F16, name="ident")
        make_identity(nc, ident[:])
        xg_sb = cpool.tile([W, NCH], I32, name="xg_sb")
        nc.sync.dma_start(out=xg_sb[:], in_=xg_d[:])
        if use_bias:
            ones = cpool.tile([P, W], F16, name="ones")
            nc.vector.memset(ones[:], 0.0)
            nc.vector.memset(ones[0:1, :], 1.0)
            bt_sb = cpool.tile([P, 2 * HID], F16, name="bt_sb")
            nc.vector.memset(bt_sb[:], 0.0)
            nc.sync.dma_start(out=bt_sb[0:1, 0:HID], in_=b1_d[:])
            nc.sync.dma_start(out=bt_sb[0:1, HID:2 * HID], in_=b2_d[:])

        xes, slabs, ps1, ps2 = {}, {}, {}, {}
        arch1, arch2 = {}, {}

        def emit_gather(c):
            xe = gpool.tile([W, EMB], F16, tag="xe", name=f"xe{c}")
            nc.gpsimd.indirect_dma_start(
                out=xe[:], out_offset=None, in_=emb_d[:],
                in_offset=IndirectOffsetOnAxis(
                    ap=xg_sb[:, c:c + 1], axis=0))
            xes[c] = xe

        def slab_items(c):
            slab = slpool.tile([P, KE * W], F16, tag="slab", name=f"slab{c}")
            slabs[c] = slab
            items = []
            for e in range(KE):
                def tr(e=e, c=c, slab=slab):
                    pt = tppool.tile([P, W], F16, tag="tp", name=f"tp{c}_{e}")
                    nc.tensor.transpose(out=pt[:],
                                        in_=xes[c][:, e * P:(e + 1) * P],
                                        identity=ident[0:W, 0:W])
                    nc.vector.tensor_copy(out=slab[:, e * W:(e + 1) * W],
                                          in_=pt[:])
                items.append(tr)
            return items

        def prod_items(lyr, c):
            """Production matmuls writing pre-activations for chunk c of
            layer `lyr` directly into a fresh PSUM chunk bank."""
            pool, store = (p1pool, ps1) if lyr == 1 else (p2pool, ps2)
            ps = pool.tile([P, CW], F32, tag=f"ps{lyr}", name=f"ps{lyr}_{c}")
            store[c] = ps
            wsb, nk = (wi1_sb, KE) if lyr == 1 else (wi2_sb, KH)
            has_bias = bias1 if lyr == 1 else bias2
            items = []
            for m in range(M):
                for k in range(nk):
                    def mm(m=m, k=k, c=c, lyr=lyr, ps=ps, wsb=wsb):
                        if lyr == 1:
                            rhs = slabs[c][:, k * MB:(k + 1) * MB]
                        else:
                            rhs = arch1[c][:, k * MB:(k + 1) * MB]
                        nc.tensor.matmul(
                            ps[:, m * MB:(m + 1) * MB],
                            lhsT=wsb[:, (k * M + m) * P:(k * M + m + 1) * P],
                            rhs=rhs,
                            start=(m == 0 and k == 0), stop=False,
                            skip_group_check=True)
                    items.append(mm)
                if has_bias:
                    def bm(m=m, ps=ps, lyr=lyr):
                        off = (0 if lyr == 1 else HID) + m * P
                        nc.tensor.matmul(
                            ps[:, m * MB:(m + 1) * MB],
                            lhsT=bt_sb[:, off:off + P],
                            rhs=ones[:],
                            start=False, stop=False, skip_group_check=True)
                    items.append(bm)
            return items

        def scan_step(lyr, c, t):
            apool, store, psd = ((a1pool, arch1, ps1) if lyr == 1
                                 else (a2pool, arch2, ps2))
            wsb = whs[lyr]
            ps = psd[c]
            if t == 0:
                arch = apool.tile([P, CW], F16, tag=f"arch{lyr}",
                                  name=f"arch{lyr}_{c}")
                store[c] = arch
            else:
                arch = store[c]
            first = (c == 0 and t == 0)
            if not first:
                if t > 0:
                    rsrc, rt = arch, t - 1
                else:
                    rsrc, rt = store[c - 1], CS - 1
                for k in range(KH):
                    for m in range(M):
                        nc.tensor.matmul(
                            ps[:, m * MB + t * RB:m * MB + (t + 1) * RB],
                            lhsT=wsb[:, (k * M + m) * P:(k * M + m + 1) * P],
                            rhs=rsrc[:, k * MB + rt * RB:k * MB + (rt + 1) * RB],
                            start=False,
                            stop=(k == KH - 1 and m == M - 1),
                            skip_group_check=True)
            nc.scalar.activation(
                out=_mv(arch[:])[:, :, t * RB:(t + 1) * RB],
                in_=_mv(ps[:])[:, :, t * RB:(t + 1) * RB], func=AF.Tanh)

        # ---- prologue: gathers first, then weights, chunk-0 slab + prod ----
        emit_gather(0)
        emit_gather(1)
        wd_sb = cpool.tile([P, M], F16, name="wd_sb")
        nc.sync.dma_start(out=wd_sb[:], in_=wd_d[:])
        bd_sb = cpool.tile([P, 1], F32, name="bd_sb")
        nc.sync.dma_start(out=bd_sb[0:RB, 0:1], in_=bd_d[:])
        whs = {}
        wsb1 = wpool.tile([P, KH * HID], F16, name="wh1_sb")
        for k in range(KH):
            nc.sync.dma_start(out=wsb1[:, k * HID:(k + 1) * HID],
                              in_=wh1_d[k * P:(k + 1) * P, :])
        whs[1] = wsb1
        wi1_sb = wpool.tile([P, KE * HID], F16, name="wi1_sb")
        for k in range(KE):
            nc.sync.dma_start(out=wi1_sb[:, k * HID:(k + 1) * HID],
                              in_=wi1_d[k * P:(k + 1) * P, :])
        wi2_sb = wpool.tile([P, KH * HID], F16, name="wi2_sb")
        for k in range(KH):
            nc.sync.dma_start(out=wi2_sb[:, k * HID:(k + 1) * HID],
                              in_=wi2_d[k * P:(k + 1) * P, :])
        wsb2 = wpool.tile([P, KH * HID], F16, name="wh2_sb")
        for k in range(KH):
            nc.sync.dma_start(out=wsb2[:, k * HID:(k + 1) * HID],
                              in_=wh2_d[k * P:(k + 1) * P, :])
        whs[2] = wsb2
        for it in slab_items(0):
            it()
        for it in prod_items(1, 0):
            it()

        # ---- main pipelined windows ----
        for w in range(NCH + LAG if nwin is None else nwin):
            items = []
            if w + 2 <= NCH - 1:
                items.append(lambda c=w + 2: emit_gather(c))
            if w + 1 <= NCH - 1:
                items += slab_items(w + 1)
                items += prod_items(1, w + 1)
            if 0 <= w - 1 <= NCH - 1:
                items += prod_items(2, w - 1)
            budget = (len(items) + 9) // 10
            for t in range(CS):
                if w <= NCH - 1:
                    scan_step(1, w, t)
                    for _ in range(budget):
                        if items:
                            items.pop(0)()
                if 0 <= w - LAG <= NCH - 1:
                    scan_step(2, w - LAG, t)
                    for _ in range(budget):
                        if items:
                            items.pop(0)()
            for it in items:
                it()

        # ---- head: y = sigmoid(h2_last @ Wd + bd) ----
        hps = p1pool.tile([RB, 1], F32, tag="ps1", name="hps")
        last = arch2[max(k for k in arch2)] if arch2 else arch1[max(k for k in arch1)]
        for m in range(M):
            nc.tensor.matmul(
                hps[:], lhsT=last[:, m * MB + (CS - 1) * RB:
                                 m * MB + CS * RB],
                rhs=wd_sb[:, m:m + 1], start=(m == 0), stop=(m == M - 1))
        y_sb = cpool.tile([P, 1], F32, name="y_sb")
        nc.scalar.activation(out=y_sb[0:RB, 0:1], in_=hps[:],
                             func=AF.Tanh, scale=0.5, bias=bd_sb[0:RB, 0:1])
        y2_sb = cpool.tile([P, 1], F32, name="y2_sb")
        nc.vector.tensor_scalar(out=y2_sb[0:RB, 0:1], in0=y_sb[0:RB, 0:1],
                                scalar1=0.5, scalar2=0.5,
                                op0=mybir.AluOpType.mult,
                                op1=mybir.AluOpType.add)
        nc.sync.dma_start(out=y_d[:], in_=y2_sb[0:RB, 0:1])

    prune_redundant_self_waits(nc)
    nc.compile()
    return nc


def _prep_maps(x, emb, Wi1, Wh1, b1, Wi2, Wh2, b2, Wd, bd):
    x = np.asarray(x, np.int64)
    shared = {
        "emb": np.ascontiguousarray(np.asarray(emb, NPF)),
        "wh1": np.ascontiguousarray(np.asarray(Wh1, NPF)),
        "wh2": np.ascontiguousarray(np.asarray(Wh2, NPF)),
        "wi1": np.ascontiguousarray(np.asarray(Wi1, NPF)),
        "wi2": np.ascontiguousarray(np.asarray(Wi2, NPF)),
        "b1t": np.ascontiguousarray(np.asarray(b1, NPF).reshape(1, HID)),
        "b2t": np.ascontiguousarray(np.asarray(b2, NPF).reshape(1, HID)),
        "wdk": np.ascontiguousarray(np.asarray(Wd, NPF).reshape(M, P).T),
        "bdv": np.ascontiguousarray(
            np.broadcast_to(np.asarray(bd, np.float32) * 0.5, (RB,))),
    }
    in_maps = []
    for c in range(NCORES):
        xs = x[c * RB:(c + 1) * RB, :]                          # [8, 512]
        tok = np.ascontiguousarray(xs.T).reshape(NCH, CS * RB)  # (t, b) order
        xg = np.ascontiguousarray(tok.T.astype(np.int32))  # [64, NCH]
        in_maps.append({**shared, "xg": xg})
    return in_maps


def kernel(x, emb, Wi1, Wh1, b1, Wi2, Wh2, b2, Wd, bd):
    bias1 = bool(np.any(np.asarray(b1)))
    bias2 = bool(np.any(np.asarray(b2)))
    key = (bias1, bias2)
    if key not in _BUILT:
        _BUILT[key] = build(bias1=bias1, bias2=bias2)
    nc = _BUILT[key]
    in_maps = _prep_maps(x, emb, Wi1, Wh1, b1, Wi2, Wh2, b2, Wd, bd)
    res = run_bass_kernel_spmd(nc, in_maps, list(range(NCORES)))
    kernel.last_result = res
    y = np.concatenate([np.asarray(res.results[c]["y"], np.float32)
                        for c in range(NCORES)])
    return y


# revision 7
# speedup vs baseline: 1.2788x; 1.0101x over previous
"""Two-layer Elman RNN (B=64, S=512, EMB=512, HID=1024) on 8 TRN2 NeuronCores.

Pure data-parallel layout: core c owns batch rows [8c, 8c+8) and runs BOTH
layers itself as two software-pipelined recurrence chains — the layer-1 chain
at chunk k and the layer-2 chain LAG=2 chunks behind — so no collectives and
no DRAM round-trip for h1 are needed; the L2 chain's scan input is produced
locally from the L1 arch in SBUF.

Per chain-step the whole h update is ONE PSUM accumulation group in a
chunk-sized PSUM bank: the production pass (x@Wi1 for L1, h1@Wi2 for L2,
emitted a window ahead) writes the pre-activations for all CS=8 steps of the
chunk straight into the bank (first matmul start=True zero-arms the bank;
later first-touch writes auto-zero, subsequent writes accumulate), the 64
per-step Wh tile matmuls accumulate on top, and a single [128, 64] Tanh
drains the step's columns into the fp16 h archive. The critical chain per
step is tanh -> 64 matmuls -> tanh with one cross-engine semaphore hop each
direction; prune_redundant_self_waits removes tile's redundant self-engine
tick waits so dependent instructions carry their cross-engine wait directly
instead of behind a blocking EventSemaphore. The two chains' steps
interleave, filling each chain's latency window with the other chain's work
plus production/gather/transpose items.
"""

import collections
import re
from contextlib import ExitStack

import numpy as np

import concourse.bass as bass
import concourse.bacc as bacc
import concourse.mybir as mybir
import concourse.tile as tile
from concourse.bass import IndirectOffsetOnAxis
from concourse.bass_utils import run_bass_kernel_spmd
from concourse.masks import make_identity

P = 128
VOCAB, EMB, HID = 50257, 512, 1024
B, S = 64, 512
NCORES = 8
RB = B // NCORES          # 8 batch rows per core
M = HID // P              # 8 output feature blocks
KH = HID // P             # 8 contraction tiles (hidden)
KE = EMB // P             # 4 contraction tiles (embedding)
CS = 8                    # scan steps per chunk
NCH = S // CS             # 64 chunks
LAG = 2                   # chunks the L2 chain trails the L1 chain
W = M * RB                # 64 (m, b) columns per step
CW = CS * W               # 512 columns per chunk

F16 = mybir.dt.float16
F32 = mybir.dt.float32
I32 = mybir.dt.int32
NPF = np.float16

_BUILT = {}

_TICK = re.compile(r"^(PE|Activation|DVE|Pool|SP)_\d+$")


def prune_redundant_self_waits(nc):
    """Drop waits provably satisfied by same-engine program order.

    Tile assigns every instruction a wait on its own engine's tick semaphore;
    on TRN2 each instruction may carry at most ONE wait, so any cross-engine
    dependency then gets split into a separate blocking EventSemaphore that
    serializes the sequencer behind the wait and adds a post-release decode
    to the critical chain.  A wait `S >= V` on engine E is redundant iff S is
    E's own tick sem (only E's instructions increment it, once each, in
    order) and >= V increments precede this instruction in the same block.
    DMA-completion sems (DMAHW*/SWDGE) never match the tick-sem pattern.
    """
    for fn in nc.m.functions:
        for bb in fn.blocks:
            inc_count = collections.Counter()
            for inst in bb.instructions:
                si = inst.sync_info
                eng = getattr(inst.engine, "value", str(inst.engine))
                if si is not None and si.on_wait:
                    keep = []
                    for wt in si.on_wait:
                        mt = _TICK.match(wt.ant_name or "")
                        if (
                            mt is not None
                            and mt.group(1) == eng
                            and wt.sync_type == "semaphore"
                            and wt.wait_mode == "sem-ge-imm"
                            and wt.wait_reg is None
                            and inc_count.get(wt.ant_name, 0) >= wt.wait_value
                        ):
                            continue
                        keep.append(wt)
                    if len(keep) != len(si.on_wait):
                        inst.sync_info = type(si)(
                            on_wait=keep, on_update=list(si.on_update)
                        )
                si = inst.sync_info
                if si is not None:
                    for u in si.on_update:
                        if (
                            u.sync_type == "semaphore"
                            and u.update_mode == "sem-inc"
                            and _TICK.match(u.ant_name or "")
                        ):
                            inc_count[u.ant_name] += u.update_value


MB = CS * RB              # 64 cols per m-block in the (m, t, b) chunk layout


def _mv(ap):
    """View a [P, CW] chunk AP as [P, m, tb] with tb = MB cols per m-block."""
    return ap.rearrange("p (m tb) -> p m tb", tb=MB)


def build(local_cc=False, bias1=False, bias2=False, nwin=None):
    del local_cc  # no collectives in this kernel; kept for test harness compat
    nc = bacc.Bacc("TRN2", target_bir_lowering=False, debug=False,
                   num_devices=NCORES)

    xg_d = nc.dram_tensor("xg", [W, NCH], I32, kind="ExternalInput").ap()
    emb_d = nc.dram_tensor("emb", [VOCAB, EMB], F16, kind="ExternalInput").ap()
    wh1_d = nc.dram_tensor("wh1", [HID, HID], F16, kind="ExternalInput").ap()
    wh2_d = nc.dram_tensor("wh2", [HID, HID], F16, kind="ExternalInput").ap()
    wi1_d = nc.dram_tensor("wi1", [EMB, HID], F16, kind="ExternalInput").ap()
    wi2_d = nc.dram_tensor("wi2", [HID, HID], F16, kind="ExternalInput").ap()
    b1_d = nc.dram_tensor("b1t", [1, HID], F16, kind="ExternalInput").ap()
    b2_d = nc.dram_tensor("b2t", [1, HID], F16, kind="ExternalInput").ap()
    wd_d = nc.dram_tensor("wdk", [P, M], F16, kind="ExternalInput").ap()
    bd_d = nc.dram_tensor("bdv", [RB], F32, kind="ExternalInput").ap()
    y_d = nc.dram_tensor("y", [RB], F32, kind="ExternalOutput").ap()

    AF = mybir.ActivationFunctionType
    use_bias = bias1 or bias2

    with tile.TileContext(nc) as tc, ExitStack() as ctx:
        cpool = ctx.enter_context(tc.tile_pool(name="const", bufs=1))
        wpool = ctx.enter_context(tc.tile_pool(name="weights", bufs=1))
        gpool = ctx.enter_context(tc.tile_pool(name="gather", bufs=3))
        slpool = ctx.enter_context(tc.tile_pool(name="slab", bufs=2))
        a1pool = ctx.enter_context(tc.tile_pool(name="arch1", bufs=4))
        a2pool = ctx.enter_context(tc.tile_pool(name="arch2", bufs=4))
        p1pool = ctx.enter_context(tc.tile_pool(name="ps1", bufs=3, space="PSUM"))
        p2pool = ctx.enter_context(tc.tile_pool(name="ps2", bufs=3, space="PSUM"))
        tppool = ctx.enter_context(tc.tile_pool(name="tp", bufs=2, space="PSUM"))

        ident = cpool.tile([P, P], F16, name="ident")
        make_identity(nc, ident[:])
        xg_sb = cpool.tile([W, NCH], I32, name="xg_sb")
        nc.sync.dma_start(out=xg_sb[:], in_=xg_d[:])
        if use_bias:
            ones = cpool.tile([P, W], F16, name="ones")
            nc.vector.memset(ones[:], 0.0)
            nc.vector.memset(ones[0:1, :], 1.0)
            bt_sb = cpool.tile([P, 2 * HID], F16, name="bt_sb")
            nc.vector.memset(bt_sb[:], 0.0)
            nc.sync.dma_start(out=bt_sb[0:1, 0:HID], in_=b1_d[:])
            nc.sync.dma_start(out=bt_sb[0:1, HID:2 * HID], in_=b2_d[:])

        xes, slabs, ps1, ps2 = {}, {}, {}, {}
        arch1, arch2 = {}, {}

        def emit_gather(c):
            xe = gpool.tile([W, EMB], F16, tag="xe", name=f"xe{c}")
            nc.gpsimd.indirect_dma_start(
                out=xe[:], out_offset=None, in_=emb_d[:],
                in_offset=IndirectOffsetOnAxis(
                    ap=xg_sb[:, c:c + 1], axis=0))
            xes[c] = xe

        def slab_items(c):
            slab = slpool.tile([P, KE * W], F16, tag="slab", name=f"slab{c}")
            slabs[c] = slab
            items = []
            for e in range(KE):
                def tr(e=e, c=c, slab=slab):
                    pt = tppool.tile([P, W], F16, tag="tp", name=f"tp{c}_{e}")
                    nc.tensor.transpose(out=pt[:],
                                        in_=xes[c][:, e * P:(e + 1) * P],
                                        identity=ident[0:W, 0:W])
                    nc.vector.tensor_copy(out=slab[:, e * W:(e + 1) * W],
                                          in_=pt[:])
                items.append(tr)
            return items

        def prod_items(lyr, c):
            """Production matmuls writing pre-activations for chunk c of
            layer `lyr` directly into a fresh PSUM chunk bank."""
            pool, store = (p1pool, ps1) if lyr == 1 else (p2pool, ps2)
            ps = pool.tile([P, CW], F32, tag=f"ps{lyr}", name=f"ps{lyr}_{c}")
            store[c] = ps
            wsb, nk = (wi1_sb, KE) if lyr == 1 else (wi2_sb, KH)
            has_bias = bias1 if lyr == 1 else bias2
            items = []
            for m in range(M):
                for k in range(nk):
                    def mm(m=m, k=k, c=c, lyr=lyr, ps=ps, wsb=wsb):
                        if lyr == 1:
                            rhs = slabs[c][:, k * MB:(k + 1) * MB]
                        else:
                            rhs = arch1[c][:, k * MB:(k + 1) * MB]
                        nc.tensor.matmul(
                            ps[:, m * MB:(m + 1) * MB],
                            lhsT=wsb[:, (k * M + m) * P:(k * M + m + 1) * P],
                            rhs=rhs,
                            start=(m == 0 and k == 0), stop=False,
                            skip_group_check=True)
                    items.append(mm)
                if has_bias:
                    def bm(m=m, ps=ps, lyr=lyr):
                        off = (0 if lyr == 1 else HID) + m * P
                        nc.tensor.matmul(
                            ps[:, m * MB:(m + 1) * MB],
                            lhsT=bt_sb[:, off:off + P],
                            rhs=ones[:],
                            start=False, stop=False, skip_group_check=True)
                    items.append(bm)
            return items

        def scan_step(lyr, c, t):
            apool, store, psd = ((a1pool, arch1, ps1) if lyr == 1
                                 else (a2pool, arch2, ps2))
            wsb = whs[lyr]
            ps = psd[c]
            if t == 0:
                arch = apool.tile([P, CW], F16, tag=f"arch{lyr}",
                                  name=f"arch{lyr}_{c}")
                store[c] = arch
            else:
                arch = store[c]
            first = (c == 0 and t == 0)
            if not first:
                if t > 0:
                    rsrc, rt = arch, t - 1
                else:
                    rsrc, rt = store[c - 1], CS - 1
                for k in range(KH):
                    for m in range(M):
                        nc.tensor.matmul(
                            ps[:, m * MB + t * RB:m * MB + (t + 1) * RB],
                            lhsT=wsb[:, (k * M + m) * P:(k * M + m + 1) * P],
                            rhs=rsrc[:, k * MB + rt * RB:k * MB + (rt + 1) * RB],
                            start=False,
                            stop=(k == KH - 1 and m == M - 1),
                            skip_group_check=True)
            nc.scalar.activation(
                out=_mv(arch[:])[:, :, t * RB:(t + 1) * RB],
                in_=_mv(ps[:])[:, :, t * RB:(t + 1) * RB], func=AF.Tanh)

        # ---- prologue: gathers first, then weights, chunk-0 slab + prod ----
        emit_gather(0)
        emit_gather(1)
        wd_sb = cpool.tile([P, M], F16, name="wd_sb")
        nc.sync.dma_start(out=wd_sb[:], in_=wd_d[:])
        bd_sb = cpool.tile([P, 1], F32, name="bd_sb")
        nc.sync.dma_start(out=bd_sb[0:RB, 0:1], in_=bd_d[:])
        whs = {}
        wsb1 = wpool.tile([P, KH * HID], F16, name="wh1_sb")
        for k in range(KH):
            nc.sync.dma_start(out=wsb1[:, k * HID:(k + 1) * HID],
                              in_=wh1_d[k * P:(k + 1) * P, :])
        whs[1] = wsb1
        wi1_sb = wpool.tile([P, KE * HID], F16, name="wi1_sb")
        for k in range(KE):
            nc.sync.dma_start(out=wi1_sb[:, k * HID:(k + 1) * HID],
                              in_=wi1_d[k * P:(k + 1) * P, :])
        wi2_sb = wpool.tile([P, KH * HID], F16, name="wi2_sb")
        for k in range(KH):
            nc.sync.dma_start(out=wi2_sb[:, k * HID:(k + 1) * HID],
                              in_=wi2_d[k * P:(k + 1) * P, :])
        wsb2 = wpool.tile([P, KH * HID], F16, name="wh2_sb")
        for k in range(KH):
            nc.sync.dma_start(out=wsb2[:, k * HID:(k + 1) * HID],
                              in_=wh2_d[k * P:(k + 1) * P, :])
        whs[2] = wsb2
        for it in slab_items(0):
            it()
        for it in prod_items(1, 0):
            it()

        # ---- main pipelined windows ----
        for w in range(NCH + LAG if nwin is None else nwin):
            items = []
            if w + 2 <= NCH - 1:
                items.append(lambda c=w + 2: emit_gather(c))
            if w + 1 <= NCH - 1:
                items += slab_items(w + 1)
                items += prod_items(1, w + 1)
            if 0 <= w - 1 <= NCH - 1:
                items += prod_items(2, w - 1)
            budget = (len(items) + 13) // 14
            for t in range(CS):
                if w <= NCH - 1:
                    scan_step(1, w, t)
                    for _ in range(budget):
                        if items:
                            items.pop(0)()
                if 0 <= w - LAG <= NCH - 1:
                    scan_step(2, w - LAG, t)
                    for _ in range(budget):
                        if items:
                            items.pop(0)()
            for it in items:
                it()

        # ---- head: y = sigmoid(h2_last @ Wd + bd) ----
        hps = p1pool.tile([RB, 1], F32, tag="ps1", name="hps")
        last = arch2[max(k for k in arch2)] if arch2 else arch1[max(k for k in arch1)]
        for m in range(M):
            nc.tensor.matmul(
                hps[:], lhsT=last[:, m * MB + (CS - 1) * RB:
                                 m * MB + CS * RB],
                rhs=wd_sb[:, m:m + 1], start=(m == 0), stop=(m == M - 1))
        y_sb = cpool.tile([P, 1], F32, name="y_sb")
        nc.scalar.activation(out=y_sb[0:RB, 0:1], in_=hps[:],
                             func=AF.Tanh, scale=0.5, bias=bd_sb[0:RB, 0:1])
        y2_sb = cpool.tile([P, 1], F32, name="y2_sb")
        nc.vector.tensor_scalar(out=y2_sb[0:RB, 0:1], in0=y_sb[0:RB, 0:1],
                                scalar1=0.5, scalar2=0.5,
                                op0=mybir.AluOpType.mult,
                                op1=mybir.AluOpType.add)
        nc.sync.dma_start(out=y_d[:], in_=y2_sb[0:RB, 0:1])

    prune_redundant_self_waits(nc)
    nc.compile()
    return nc


def _prep_maps(x, emb, Wi1, Wh1, b1, Wi2, Wh2, b2, Wd, bd):
    x = np.asarray(x, np.int64)
    shared = {
        "emb": np.ascontiguousarray(np.asarray(emb, NPF)),
        "wh1": np.ascontiguousarray(np.asarray(Wh1, NPF)),
        "wh2": np.ascontiguousarray(np.asarray(Wh2, NPF)),
        "wi1": np.ascontiguousarray(np.asarray(Wi1, NPF)),
        "wi2": np.ascontiguousarray(np.asarray(Wi2, NPF)),
        "b1t": np.ascontiguousarray(np.asarray(b1, NPF).reshape(1, HID)),
        "b2t": np.ascontiguousarray(np.asarray(b2, NPF).reshape(1, HID)),
        "wdk": np.ascontiguousarray(np.asarray(Wd, NPF).reshape(M, P).T),
        "bdv": np.ascontiguousarray(
            np.broadcast_to(np.asarray(bd, np.float32) * 0.5, (RB,))),
    }
    in_maps = []
    for c in range(NCORES):
        xs = x[c * RB:(c + 1) * RB, :]                          # [8, 512]
        tok = np.ascontiguousarray(xs.T).reshape(NCH, CS * RB)  # (t, b) order
        xg = np.ascontiguousarray(tok.T.astype(np.int32))  # [64, NCH]
        in_maps.append({**shared, "xg": xg})
    return in_maps


def kernel(x, emb, Wi1, Wh1, b1, Wi2, Wh2, b2, Wd, bd):
    bias1 = bool(np.any(np.asarray(b1)))
    bias2 = bool(np.any(np.asarray(b2)))
    key = (bias1, bias2)
    if key not in _BUILT:
        _BUILT[key] = build(bias1=bias1, bias2=bias2)
    nc = _BUILT[key]
    in_maps = _prep_maps(x, emb, Wi1, Wh1, b1, Wi2, Wh2, b2, Wd, bd)
    res = run_bass_kernel_spmd(nc, in_maps, list(range(NCORES)))
    kernel.last_result = res
    y = np.concatenate([np.asarray(res.results[c]["y"], np.float32)
                        for c in range(NCORES)])
    return y


# revision 8
# speedup vs baseline: 1.2827x; 1.0031x over previous
"""Two-layer Elman RNN (B=64, S=512, EMB=512, HID=1024) on 8 TRN2 NeuronCores.

Pure data-parallel layout: core c owns batch rows [8c, 8c+8) and runs BOTH
layers itself as two software-pipelined recurrence chains — the layer-1 chain
at chunk k and the layer-2 chain LAG=2 chunks behind — so no collectives and
no DRAM round-trip for h1 are needed; the L2 chain's scan input is produced
locally from the L1 arch in SBUF.

Per chain-step the whole h update is ONE PSUM accumulation group in a
chunk-sized PSUM bank: the production pass (x@Wi1 for L1, h1@Wi2 for L2,
emitted a window ahead) writes the pre-activations for all CS=8 steps of the
chunk straight into the bank (first matmul start=True zero-arms the bank;
later first-touch writes auto-zero, subsequent writes accumulate), the 64
per-step Wh tile matmuls accumulate on top, and a single [128, 64] Tanh
drains the step's columns into the fp16 h archive. The critical chain per
step is tanh -> 64 matmuls -> tanh with one cross-engine semaphore hop each
direction; prune_redundant_self_waits removes tile's redundant self-engine
tick waits so dependent instructions carry their cross-engine wait directly
instead of behind a blocking EventSemaphore. The two chains' steps
interleave, filling each chain's latency window with the other chain's work
plus production/gather/transpose items.
"""

import collections
import re
from contextlib import ExitStack

import numpy as np

import concourse.bass as bass
import concourse.bacc as bacc
import concourse.mybir as mybir
import concourse.tile as tile
from concourse.bass import IndirectOffsetOnAxis
from concourse.bass_utils import run_bass_kernel_spmd
from concourse.masks import make_identity

P = 128
VOCAB, EMB, HID = 50257, 512, 1024
B, S = 64, 512
NCORES = 8
RB = B // NCORES          # 8 batch rows per core
M = HID // P              # 8 output feature blocks
KH = HID // P             # 8 contraction tiles (hidden)
KE = EMB // P             # 4 contraction tiles (embedding)
CS = 8                    # scan steps per chunk
NCH = S // CS             # 64 chunks
LAG = 2                   # chunks the L2 chain trails the L1 chain
W = M * RB                # 64 (m, b) columns per step
CW = CS * W               # 512 columns per chunk

F16 = mybir.dt.float16
F32 = mybir.dt.float32
I32 = mybir.dt.int32
NPF = np.float16

_BUILT = {}

_TICK = re.compile(r"^(PE|Activation|DVE|Pool|SP)_\d+$")


def prune_redundant_self_waits(nc):
    """Drop waits provably satisfied by same-engine program order.

    Tile assigns every instruction a wait on its own engine's tick semaphore;
    on TRN2 each instruction may carry at most ONE wait, so any cross-engine
    dependency then gets split into a separate blocking EventSemaphore that
    serializes the sequencer behind the wait and adds a post-release decode
    to the critical chain.  A wait `S >= V` on engine E is redundant iff S is
    E's own tick sem (only E's instructions increment it, once each, in
    order) and >= V increments precede this instruction in the same block.
    DMA-completion sems (DMAHW*/SWDGE) never match the tick-sem pattern.
    """
    for fn in nc.m.functions:
        for bb in fn.blocks:
            inc_count = collections.Counter()
            for inst in bb.instructions:
                si = inst.sync_info
                eng = getattr(inst.engine, "value", str(inst.engine))
                if si is not None and si.on_wait:
                    keep = []
                    for wt in si.on_wait:
                        mt = _TICK.match(wt.ant_name or "")
                        if (
                            mt is not None
                            and mt.group(1) == eng
                            and wt.sync_type == "semaphore"
                            and wt.wait_mode == "sem-ge-imm"
                            and wt.wait_reg is None
                            and inc_count.get(wt.ant_name, 0) >= wt.wait_value
                        ):
                            continue
                        keep.append(wt)
                    if len(keep) != len(si.on_wait):
                        inst.sync_info = type(si)(
                            on_wait=keep, on_update=list(si.on_update)
                        )
                si = inst.sync_info
                if si is not None:
                    for u in si.on_update:
                        if (
                            u.sync_type == "semaphore"
                            and u.update_mode == "sem-inc"
                            and _TICK.match(u.ant_name or "")
                        ):
                            inc_count[u.ant_name] += u.update_value


MB = CS * RB              # 64 cols per m-block in the (m, t, b) chunk layout


def _mv(ap):
    """View a [P, CW] chunk AP as [P, m, tb] with tb = MB cols per m-block."""
    return ap.rearrange("p (m tb) -> p m tb", tb=MB)


def build(local_cc=False, bias1=False, bias2=False, nwin=None):
    del local_cc  # no collectives in this kernel; kept for test harness compat
    nc = bacc.Bacc("TRN2", target_bir_lowering=False, debug=False,
                   num_devices=NCORES)

    xg_d = nc.dram_tensor("xg", [W, NCH], I32, kind="ExternalInput").ap()
    emb_d = nc.dram_tensor("emb", [VOCAB, EMB], F16, kind="ExternalInput").ap()
    wh1_d = nc.dram_tensor("wh1", [HID, HID], F16, kind="ExternalInput").ap()
    wh2_d = nc.dram_tensor("wh2", [HID, HID], F16, kind="ExternalInput").ap()
    wi1_d = nc.dram_tensor("wi1", [EMB, HID], F16, kind="ExternalInput").ap()
    wi2_d = nc.dram_tensor("wi2", [HID, HID], F16, kind="ExternalInput").ap()
    b1_d = nc.dram_tensor("b1t", [1, HID], F16, kind="ExternalInput").ap()
    b2_d = nc.dram_tensor("b2t", [1, HID], F16, kind="ExternalInput").ap()
    wd_d = nc.dram_tensor("wdk", [P, M], F16, kind="ExternalInput").ap()
    bd_d = nc.dram_tensor("bdv", [RB], F32, kind="ExternalInput").ap()
    y_d = nc.dram_tensor("y", [RB], F32, kind="ExternalOutput").ap()

    AF = mybir.ActivationFunctionType
    use_bias = bias1 or bias2

    with tile.TileContext(nc) as tc, ExitStack() as ctx:
        cpool = ctx.enter_context(tc.tile_pool(name="const", bufs=1))
        wpool = ctx.enter_context(tc.tile_pool(name="weights", bufs=1))
        gpool = ctx.enter_context(tc.tile_pool(name="gather", bufs=6))
        slpool = ctx.enter_context(tc.tile_pool(name="slab", bufs=2))
        a1pool = ctx.enter_context(tc.tile_pool(name="arch1", bufs=4))
        a2pool = ctx.enter_context(tc.tile_pool(name="arch2", bufs=4))
        p1pool = ctx.enter_context(tc.tile_pool(name="ps1", bufs=3, space="PSUM"))
        p2pool = ctx.enter_context(tc.tile_pool(name="ps2", bufs=3, space="PSUM"))
        tppool = ctx.enter_context(tc.tile_pool(name="tp", bufs=2, space="PSUM"))

        ident = cpool.tile([P, P], F16, name="ident")
        make_identity(nc, ident[:])
        xg_sb = cpool.tile([W, NCH], I32, name="xg_sb")
        nc.sync.dma_start(out=xg_sb[:], in_=xg_d[:])
        if use_bias:
            ones = cpool.tile([P, W], F16, name="ones")
            nc.vector.memset(ones[:], 0.0)
            nc.vector.memset(ones[0:1, :], 1.0)
            bt_sb = cpool.tile([P, 2 * HID], F16, name="bt_sb")
            nc.vector.memset(bt_sb[:], 0.0)
            nc.sync.dma_start(out=bt_sb[0:1, 0:HID], in_=b1_d[:])
            nc.sync.dma_start(out=bt_sb[0:1, HID:2 * HID], in_=b2_d[:])

        xes, slabs, ps1, ps2 = {}, {}, {}, {}
        arch1, arch2 = {}, {}

        def emit_gather(c):
            xe = gpool.tile([W, EMB], F16, tag="xe", name=f"xe{c}")
            nc.gpsimd.indirect_dma_start(
                out=xe[:], out_offset=None, in_=emb_d[:],
                in_offset=IndirectOffsetOnAxis(
                    ap=xg_sb[:, c:c + 1], axis=0))
            xes[c] = xe

        def slab_items(c):
            slab = slpool.tile([P, KE * W], F16, tag="slab", name=f"slab{c}")
            slabs[c] = slab
            items = []
            for e in range(KE):
                def tr(e=e, c=c, slab=slab):
                    pt = tppool.tile([P, W], F16, tag="tp", name=f"tp{c}_{e}")
                    nc.tensor.transpose(out=pt[:],
                                        in_=xes[c][:, e * P:(e + 1) * P],
                                        identity=ident[0:W, 0:W])
                    nc.vector.tensor_copy(out=slab[:, e * W:(e + 1) * W],
                                          in_=pt[:])
                items.append(tr)
            return items

        def prod_items(lyr, c):
            """Production matmuls writing pre-activations for chunk c of
            layer `lyr` directly into a fresh PSUM chunk bank."""
            pool, store = (p1pool, ps1) if lyr == 1 else (p2pool, ps2)
            ps = pool.tile([P, CW], F32, tag=f"ps{lyr}", name=f"ps{lyr}_{c}")
            store[c] = ps
            wsb, nk = (wi1_sb, KE) if lyr == 1 else (wi2_sb, KH)
            has_bias = bias1 if lyr == 1 else bias2
            items = []
            for m in range(M):
                for k in range(nk):
                    def mm(m=m, k=k, c=c, lyr=lyr, ps=ps, wsb=wsb):
                        if lyr == 1:
                            rhs = slabs[c][:, k * MB:(k + 1) * MB]
                        else:
                            rhs = arch1[c][:, k * MB:(k + 1) * MB]
                        nc.tensor.matmul(
                            ps[:, m * MB:(m + 1) * MB],
                            lhsT=wsb[:, (k * M + m) * P:(k * M + m + 1) * P],
                            rhs=rhs,
                            start=(m == 0 and k == 0), stop=False,
                            skip_group_check=True)
                    items.append(mm)
                if has_bias:
                    def bm(m=m, ps=ps, lyr=lyr):
                        off = (0 if lyr == 1 else HID) + m * P
                        nc.tensor.matmul(
                            ps[:, m * MB:(m + 1) * MB],
                            lhsT=bt_sb[:, off:off + P],
                            rhs=ones[:],
                            start=False, stop=False, skip_group_check=True)
                    items.append(bm)
            return items

        def scan_step(lyr, c, t):
            apool, store, psd = ((a1pool, arch1, ps1) if lyr == 1
                                 else (a2pool, arch2, ps2))
            wsb = whs[lyr]
            ps = psd[c]
            if t == 0:
                arch = apool.tile([P, CW], F16, tag=f"arch{lyr}",
                                  name=f"arch{lyr}_{c}")
                store[c] = arch
            else:
                arch = store[c]
            first = (c == 0 and t == 0)
            if not first:
                if t > 0:
                    rsrc, rt = arch, t - 1
                else:
                    rsrc, rt = store[c - 1], CS - 1
                for k in range(KH):
                    for m in range(M):
                        nc.tensor.matmul(
                            ps[:, m * MB + t * RB:m * MB + (t + 1) * RB],
                            lhsT=wsb[:, (k * M + m) * P:(k * M + m + 1) * P],
                            rhs=rsrc[:, k * MB + rt * RB:k * MB + (rt + 1) * RB],
                            start=False,
                            stop=(k == KH - 1 and m == M - 1),
                            skip_group_check=True)
            nc.scalar.activation(
                out=_mv(arch[:])[:, :, t * RB:(t + 1) * RB],
                in_=_mv(ps[:])[:, :, t * RB:(t + 1) * RB], func=AF.Tanh)

        # ---- prologue: gathers first, then weights, chunk-0 slab + prod ----
        emit_gather(0)
        emit_gather(1)
        wd_sb = cpool.tile([P, M], F16, name="wd_sb")
        nc.sync.dma_start(out=wd_sb[:], in_=wd_d[:])
        bd_sb = cpool.tile([P, 1], F32, name="bd_sb")
        nc.sync.dma_start(out=bd_sb[0:RB, 0:1], in_=bd_d[:])
        whs = {}
        wsb1 = wpool.tile([P, KH * HID], F16, name="wh1_sb")
        for k in range(KH):
            nc.sync.dma_start(out=wsb1[:, k * HID:(k + 1) * HID],
                              in_=wh1_d[k * P:(k + 1) * P, :])
        whs[1] = wsb1
        wi1_sb = wpool.tile([P, KE * HID], F16, name="wi1_sb")
        for k in range(KE):
            nc.sync.dma_start(out=wi1_sb[:, k * HID:(k + 1) * HID],
                              in_=wi1_d[k * P:(k + 1) * P, :])
        wi2_sb = wpool.tile([P, KH * HID], F16, name="wi2_sb")
        for k in range(KH):
            nc.sync.dma_start(out=wi2_sb[:, k * HID:(k + 1) * HID],
                              in_=wi2_d[k * P:(k + 1) * P, :])
        wsb2 = wpool.tile([P, KH * HID], F16, name="wh2_sb")
        for k in range(KH):
            nc.sync.dma_start(out=wsb2[:, k * HID:(k + 1) * HID],
                              in_=wh2_d[k * P:(k + 1) * P, :])
        whs[2] = wsb2
        for it in slab_items(0):
            it()
        for it in prod_items(1, 0):
            it()

        # ---- main pipelined windows ----
        for w in range(NCH + LAG if nwin is None else nwin):
            items = []
            if w + 2 <= NCH - 1:
                items.append(lambda c=w + 2: emit_gather(c))
            if w + 1 <= NCH - 1:
                items += slab_items(w + 1)
                items += prod_items(1, w + 1)
            if 0 <= w - 1 <= NCH - 1:
                items += prod_items(2, w - 1)
            budget = (len(items) + 13) // 14
            for t in range(CS):
                if w <= NCH - 1:
                    scan_step(1, w, t)
                    for _ in range(budget):
                        if items:
                            items.pop(0)()
                if 0 <= w - LAG <= NCH - 1:
                    scan_step(2, w - LAG, t)
                    for _ in range(budget):
                        if items:
                            items.pop(0)()
            for it in items:
                it()

        # ---- head: y = sigmoid(h2_last @ Wd + bd) ----
        hps = p1pool.tile([RB, 1], F32, tag="ps1", name="hps")
        last = arch2[max(k for k in arch2)] if arch2 else arch1[max(k for k in arch1)]
        for m in range(M):
            nc.tensor.matmul(
                hps[:], lhsT=last[:, m * MB + (CS - 1) * RB:
                                 m * MB + CS * RB],
                rhs=wd_sb[:, m:m + 1], start=(m == 0), stop=(m == M - 1))
        y_sb = cpool.tile([P, 1], F32, name="y_sb")
        nc.scalar.activation(out=y_sb[0:RB, 0:1], in_=hps[:],
                             func=AF.Tanh, scale=0.5, bias=bd_sb[0:RB, 0:1])
        y2_sb = cpool.tile([P, 1], F32, name="y2_sb")
        nc.vector.tensor_scalar(out=y2_sb[0:RB, 0:1], in0=y_sb[0:RB, 0:1],
                                scalar1=0.5, scalar2=0.5,
                                op0=mybir.AluOpType.mult,
                                op1=mybir.AluOpType.add)
        nc.sync.dma_start(out=y_d[:], in_=y2_sb[0:RB, 0:1])

    prune_redundant_self_waits(nc)
    nc.compile()
    return nc


def _prep_maps(x, emb, Wi1, Wh1, b1, Wi2, Wh2, b2, Wd, bd):
    x = np.asarray(x, np.int64)
    shared = {
        "emb": np.ascontiguousarray(np.asarray(emb, NPF)),
        "wh1": np.ascontiguousarray(np.asarray(Wh1, NPF)),
        "wh2": np.ascontiguousarray(np.asarray(Wh2, NPF)),
        "wi1": np.ascontiguousarray(np.asarray(Wi1, NPF)),
        "wi2": np.ascontiguousarray(np.asarray(Wi2, NPF)),
        "b1t": np.ascontiguousarray(np.asarray(b1, NPF).reshape(1, HID)),
        "b2t": np.ascontiguousarray(np.asarray(b2, NPF).reshape(1, HID)),
        "wdk": np.ascontiguousarray(np.asarray(Wd, NPF).reshape(M, P).T),
        "bdv": np.ascontiguousarray(
            np.broadcast_to(np.asarray(bd, np.float32) * 0.5, (RB,))),
    }
    in_maps = []
    for c in range(NCORES):
        xs = x[c * RB:(c + 1) * RB, :]                          # [8, 512]
        tok = np.ascontiguousarray(xs.T).reshape(NCH, CS * RB)  # (t, b) order
        xg = np.ascontiguousarray(tok.T.astype(np.int32))  # [64, NCH]
        in_maps.append({**shared, "xg": xg})
    return in_maps


def kernel(x, emb, Wi1, Wh1, b1, Wi2, Wh2, b2, Wd, bd):
    bias1 = bool(np.any(np.asarray(b1)))
    bias2 = bool(np.any(np.asarray(b2)))
    key = (bias1, bias2)
    if key not in _BUILT:
        _BUILT[key] = build(bias1=bias1, bias2=bias2)
    nc = _BUILT[key]
    in_maps = _prep_maps(x, emb, Wi1, Wh1, b1, Wi2, Wh2, b2, Wd, bd)
    res = run_bass_kernel_spmd(nc, in_maps, list(range(NCORES)))
    kernel.last_result = res
    y = np.concatenate([np.asarray(res.results[c]["y"], np.float32)
                        for c in range(NCORES)])
    return y


# revision 9
# speedup vs baseline: 1.2832x; 1.0004x over previous
"""Two-layer Elman RNN (B=64, S=512, EMB=512, HID=1024) on 8 TRN2 NeuronCores.

Pure data-parallel layout: core c owns batch rows [8c, 8c+8) and runs BOTH
layers itself as two software-pipelined recurrence chains — the layer-1 chain
at chunk k and the layer-2 chain LAG=2 chunks behind — so no collectives and
no DRAM round-trip for h1 are needed; the L2 chain's scan input is produced
locally from the L1 arch in SBUF.

Per chain-step the whole h update is ONE PSUM accumulation group in a
chunk-sized PSUM bank: the production pass (x@Wi1 for L1, h1@Wi2 for L2,
emitted a window ahead) writes the pre-activations for all CS=8 steps of the
chunk straight into the bank (first matmul start=True zero-arms the bank;
later first-touch writes auto-zero, subsequent writes accumulate), the 64
per-step Wh tile matmuls accumulate on top, and a single [128, 64] Tanh
drains the step's columns into the fp16 h archive. The critical chain per
step is tanh -> 64 matmuls -> tanh with one cross-engine semaphore hop each
direction; prune_redundant_self_waits removes tile's redundant self-engine
tick waits so dependent instructions carry their cross-engine wait directly
instead of behind a blocking EventSemaphore. The two chains' steps
interleave, filling each chain's latency window with the other chain's work
plus production/gather/transpose items.
"""

import collections
import re
from contextlib import ExitStack

import numpy as np

import concourse.bass as bass
import concourse.bacc as bacc
import concourse.mybir as mybir
import concourse.tile as tile
from concourse.bass import IndirectOffsetOnAxis
from concourse.bass_utils import run_bass_kernel_spmd
from concourse.masks import make_identity

P = 128
VOCAB, EMB, HID = 50257, 512, 1024
B, S = 64, 512
NCORES = 8
RB = B // NCORES          # 8 batch rows per core
M = HID // P              # 8 output feature blocks
KH = HID // P             # 8 contraction tiles (hidden)
KE = EMB // P             # 4 contraction tiles (embedding)
CS = 8                    # scan steps per chunk
NCH = S // CS             # 64 chunks
LAG = 2                   # chunks the L2 chain trails the L1 chain
W = M * RB                # 64 (m, b) columns per step
CW = CS * W               # 512 columns per chunk

F16 = mybir.dt.float16
F32 = mybir.dt.float32
I32 = mybir.dt.int32
NPF = np.float16

_BUILT = {}

_TICK = re.compile(r"^(PE|Activation|DVE|Pool|SP)_\d+$")


def prune_redundant_self_waits(nc):
    """Drop waits provably satisfied by same-engine program order.

    Tile assigns every instruction a wait on its own engine's tick semaphore;
    on TRN2 each instruction may carry at most ONE wait, so any cross-engine
    dependency then gets split into a separate blocking EventSemaphore that
    serializes the sequencer behind the wait and adds a post-release decode
    to the critical chain.  A wait `S >= V` on engine E is redundant iff S is
    E's own tick sem (only E's instructions increment it, once each, in
    order) and >= V increments precede this instruction in the same block.
    DMA-completion sems (DMAHW*/SWDGE) never match the tick-sem pattern.
    """
    for fn in nc.m.functions:
        for bb in fn.blocks:
            inc_count = collections.Counter()
            for inst in bb.instructions:
                si = inst.sync_info
                eng = getattr(inst.engine, "value", str(inst.engine))
                if si is not None and si.on_wait:
                    keep = []
                    for wt in si.on_wait:
                        mt = _TICK.match(wt.ant_name or "")
                        if (
                            mt is not None
                            and mt.group(1) == eng
                            and wt.sync_type == "semaphore"
                            and wt.wait_mode == "sem-ge-imm"
                            and wt.wait_reg is None
                            and inc_count.get(wt.ant_name, 0) >= wt.wait_value
                        ):
                            continue
                        keep.append(wt)
                    if len(keep) != len(si.on_wait):
                        inst.sync_info = type(si)(
                            on_wait=keep, on_update=list(si.on_update)
                        )
                si = inst.sync_info
                if si is not None:
                    for u in si.on_update:
                        if (
                            u.sync_type == "semaphore"
                            and u.update_mode == "sem-inc"
                            and _TICK.match(u.ant_name or "")
                        ):
                            inc_count[u.ant_name] += u.update_value


MB = CS * RB              # 64 cols per m-block in the (m, t, b) chunk layout


def _mv(ap):
    """View a [P, CW] chunk AP as [P, m, tb] with tb = MB cols per m-block."""
    return ap.rearrange("p (m tb) -> p m tb", tb=MB)


def build(local_cc=False, bias1=False, bias2=False, nwin=None):
    del local_cc  # no collectives in this kernel; kept for test harness compat
    nc = bacc.Bacc("TRN2", target_bir_lowering=False, debug=False,
                   num_devices=NCORES)

    xg_d = nc.dram_tensor("xg", [W, NCH], I32, kind="ExternalInput").ap()
    emb_d = nc.dram_tensor("emb", [VOCAB, EMB], F16, kind="ExternalInput").ap()
    wh1_d = nc.dram_tensor("wh1", [HID, HID], F16, kind="ExternalInput").ap()
    wh2_d = nc.dram_tensor("wh2", [HID, HID], F16, kind="ExternalInput").ap()
    wi1_d = nc.dram_tensor("wi1", [EMB, HID], F16, kind="ExternalInput").ap()
    wi2_d = nc.dram_tensor("wi2", [HID, HID], F16, kind="ExternalInput").ap()
    b1_d = nc.dram_tensor("b1t", [1, HID], F16, kind="ExternalInput").ap()
    b2_d = nc.dram_tensor("b2t", [1, HID], F16, kind="ExternalInput").ap()
    wd_d = nc.dram_tensor("wdk", [P, M], F16, kind="ExternalInput").ap()
    bd_d = nc.dram_tensor("bdv", [RB], F32, kind="ExternalInput").ap()
    y_d = nc.dram_tensor("y", [RB], F32, kind="ExternalOutput").ap()

    AF = mybir.ActivationFunctionType
    use_bias = bias1 or bias2

    with tile.TileContext(nc) as tc, ExitStack() as ctx:
        cpool = ctx.enter_context(tc.tile_pool(name="const", bufs=1))
        wpool = ctx.enter_context(tc.tile_pool(name="weights", bufs=1))
        gpool = ctx.enter_context(tc.tile_pool(name="gather", bufs=6))
        slpool = ctx.enter_context(tc.tile_pool(name="slab", bufs=4))
        a1pool = ctx.enter_context(tc.tile_pool(name="arch1", bufs=6))
        a2pool = ctx.enter_context(tc.tile_pool(name="arch2", bufs=6))
        p1pool = ctx.enter_context(tc.tile_pool(name="ps1", bufs=3, space="PSUM"))
        p2pool = ctx.enter_context(tc.tile_pool(name="ps2", bufs=3, space="PSUM"))
        tppool = ctx.enter_context(tc.tile_pool(name="tp", bufs=2, space="PSUM"))

        ident = cpool.tile([P, P], F16, name="ident")
        make_identity(nc, ident[:])
        xg_sb = cpool.tile([W, NCH], I32, name="xg_sb")
        nc.sync.dma_start(out=xg_sb[:], in_=xg_d[:])
        if use_bias:
            ones = cpool.tile([P, W], F16, name="ones")
            nc.vector.memset(ones[:], 0.0)
            nc.vector.memset(ones[0:1, :], 1.0)
            bt_sb = cpool.tile([P, 2 * HID], F16, name="bt_sb")
            nc.vector.memset(bt_sb[:], 0.0)
            nc.sync.dma_start(out=bt_sb[0:1, 0:HID], in_=b1_d[:])
            nc.sync.dma_start(out=bt_sb[0:1, HID:2 * HID], in_=b2_d[:])

        xes, slabs, ps1, ps2 = {}, {}, {}, {}
        arch1, arch2 = {}, {}

        def emit_gather(c):
            xe = gpool.tile([W, EMB], F16, tag="xe", name=f"xe{c}")
            nc.gpsimd.indirect_dma_start(
                out=xe[:], out_offset=None, in_=emb_d[:],
                in_offset=IndirectOffsetOnAxis(
                    ap=xg_sb[:, c:c + 1], axis=0))
            xes[c] = xe

        def slab_items(c):
            slab = slpool.tile([P, KE * W], F16, tag="slab", name=f"slab{c}")
            slabs[c] = slab
            items = []
            for e in range(KE):
                def tr(e=e, c=c, slab=slab):
                    pt = tppool.tile([P, W], F16, tag="tp", name=f"tp{c}_{e}")
                    nc.tensor.transpose(out=pt[:],
                                        in_=xes[c][:, e * P:(e + 1) * P],
                                        identity=ident[0:W, 0:W])
                    nc.vector.tensor_copy(out=slab[:, e * W:(e + 1) * W],
                                          in_=pt[:])
                items.append(tr)
            return items

        def prod_items(lyr, c):
            """Production matmuls writing pre-activations for chunk c of
            layer `lyr` directly into a fresh PSUM chunk bank."""
            pool, store = (p1pool, ps1) if lyr == 1 else (p2pool, ps2)
            ps = pool.tile([P, CW], F32, tag=f"ps{lyr}", name=f"ps{lyr}_{c}")
            store[c] = ps
            wsb, nk = (wi1_sb, KE) if lyr == 1 else (wi2_sb, KH)
            has_bias = bias1 if lyr == 1 else bias2
            items = []
            for m in range(M):
                for k in range(nk):
                    def mm(m=m, k=k, c=c, lyr=lyr, ps=ps, wsb=wsb):
                        if lyr == 1:
                            rhs = slabs[c][:, k * MB:(k + 1) * MB]
                        else:
                            rhs = arch1[c][:, k * MB:(k + 1) * MB]
                        nc.tensor.matmul(
                            ps[:, m * MB:(m + 1) * MB],
                            lhsT=wsb[:, (k * M + m) * P:(k * M + m + 1) * P],
                            rhs=rhs,
                            start=(m == 0 and k == 0), stop=False,
                            skip_group_check=True)
                    items.append(mm)
                if has_bias:
                    def bm(m=m, ps=ps, lyr=lyr):
                        off = (0 if lyr == 1 else HID) + m * P
                        nc.tensor.matmul(
                            ps[:, m * MB:(m + 1) * MB],
                            lhsT=bt_sb[:, off:off + P],
                            rhs=ones[:],
                            start=False, stop=False, skip_group_check=True)
                    items.append(bm)
            return items

        def scan_step(lyr, c, t):
            apool, store, psd = ((a1pool, arch1, ps1) if lyr == 1
                                 else (a2pool, arch2, ps2))
            wsb = whs[lyr]
            ps = psd[c]
            if t == 0:
                arch = apool.tile([P, CW], F16, tag=f"arch{lyr}",
                                  name=f"arch{lyr}_{c}")
                store[c] = arch
            else:
                arch = store[c]
            first = (c == 0 and t == 0)
            if not first:
                if t > 0:
                    rsrc, rt = arch, t - 1
                else:
                    rsrc, rt = store[c - 1], CS - 1
                for k in range(KH):
                    for m in range(M):
                        nc.tensor.matmul(
                            ps[:, m * MB + t * RB:m * MB + (t + 1) * RB],
                            lhsT=wsb[:, (k * M + m) * P:(k * M + m + 1) * P],
                            rhs=rsrc[:, k * MB + rt * RB:k * MB + (rt + 1) * RB],
                            start=False,
                            stop=(k == KH - 1 and m == M - 1),
                            skip_group_check=True)
            nc.scalar.activation(
                out=_mv(arch[:])[:, :, t * RB:(t + 1) * RB],
                in_=_mv(ps[:])[:, :, t * RB:(t + 1) * RB], func=AF.Tanh)

        # ---- prologue: gathers first, then weights, chunk-0 slab + prod ----
        emit_gather(0)
        emit_gather(1)
        wd_sb = cpool.tile([P, M], F16, name="wd_sb")
        nc.sync.dma_start(out=wd_sb[:], in_=wd_d[:])
        bd_sb = cpool.tile([P, 1], F32, name="bd_sb")
        nc.sync.dma_start(out=bd_sb[0:RB, 0:1], in_=bd_d[:])
        whs = {}
        wsb1 = wpool.tile([P, KH * HID], F16, name="wh1_sb")
        for k in range(KH):
            nc.sync.dma_start(out=wsb1[:, k * HID:(k + 1) * HID],
                              in_=wh1_d[k * P:(k + 1) * P, :])
        whs[1] = wsb1
        wi1_sb = wpool.tile([P, KE * HID], F16, name="wi1_sb")
        for k in range(KE):
            nc.sync.dma_start(out=wi1_sb[:, k * HID:(k + 1) * HID],
                              in_=wi1_d[k * P:(k + 1) * P, :])
        wi2_sb = wpool.tile([P, KH * HID], F16, name="wi2_sb")
        for k in range(KH):
            nc.sync.dma_start(out=wi2_sb[:, k * HID:(k + 1) * HID],
                              in_=wi2_d[k * P:(k + 1) * P, :])
        wsb2 = wpool.tile([P, KH * HID], F16, name="wh2_sb")
        for k in range(KH):
            nc.sync.dma_start(out=wsb2[:, k * HID:(k + 1) * HID],
                              in_=wh2_d[k * P:(k + 1) * P, :])
        whs[2] = wsb2
        for it in slab_items(0):
            it()
        for it in prod_items(1, 0):
            it()

        # ---- main pipelined windows ----
        for w in range(NCH + LAG if nwin is None else nwin):
            items = []
            if w + 2 <= NCH - 1:
                items.append(lambda c=w + 2: emit_gather(c))
            if w + 1 <= NCH - 1:
                items += slab_items(w + 1)
                items += prod_items(1, w + 1)
            if 0 <= w - 1 <= NCH - 1:
                items += prod_items(2, w - 1)
            budget = (len(items) + 13) // 14
            for t in range(CS):
                if w <= NCH - 1:
                    scan_step(1, w, t)
                    for _ in range(budget):
                        if items:
                            items.pop(0)()
                if 0 <= w - LAG <= NCH - 1:
                    scan_step(2, w - LAG, t)
                    for _ in range(budget):
                        if items:
                            items.pop(0)()
            for it in items:
                it()

        # ---- head: y = sigmoid(h2_last @ Wd + bd) ----
        hps = p1pool.tile([RB, 1], F32, tag="ps1", name="hps")
        last = arch2[max(k for k in arch2)] if arch2 else arch1[max(k for k in arch1)]
        for m in range(M):
            nc.tensor.matmul(
                hps[:], lhsT=last[:, m * MB + (CS - 1) * RB:
                                 m * MB + CS * RB],
                rhs=wd_sb[:, m:m + 1], start=(m == 0), stop=(m == M - 1))
        y_sb = cpool.tile([P, 1], F32, name="y_sb")
        nc.scalar.activation(out=y_sb[0:RB, 0:1], in_=hps[:],
                             func=AF.Tanh, scale=0.5, bias=bd_sb[0:RB, 0:1])
        y2_sb = cpool.tile([P, 1], F32, name="y2_sb")
        nc.vector.tensor_scalar(out=y2_sb[0:RB, 0:1], in0=y_sb[0:RB, 0:1],
                                scalar1=0.5, scalar2=0.5,
                                op0=mybir.AluOpType.mult,
                                op1=mybir.AluOpType.add)
        nc.sync.dma_start(out=y_d[:], in_=y2_sb[0:RB, 0:1])

    prune_redundant_self_waits(nc)
    nc.compile()
    return nc


def _prep_maps(x, emb, Wi1, Wh1, b1, Wi2, Wh2, b2, Wd, bd):
    x = np.asarray(x, np.int64)
    shared = {
        "emb": np.ascontiguousarray(np.asarray(emb, NPF)),
        "wh1": np.ascontiguousarray(np.asarray(Wh1, NPF)),
        "wh2": np.ascontiguousarray(np.asarray(Wh2, NPF)),
        "wi1": np.ascontiguousarray(np.asarray(Wi1, NPF)),
        "wi2": np.ascontiguousarray(np.asarray(Wi2, NPF)),
        "b1t": np.ascontiguousarray(np.asarray(b1, NPF).reshape(1, HID)),
        "b2t": np.ascontiguousarray(np.asarray(b2, NPF).reshape(1, HID)),
        "wdk": np.ascontiguousarray(np.asarray(Wd, NPF).reshape(M, P).T),
        "bdv": np.ascontiguousarray(
            np.broadcast_to(np.asarray(bd, np.float32) * 0.5, (RB,))),
    }
    in_maps = []
    for c in range(NCORES):
        xs = x[c * RB:(c + 1) * RB, :]                          # [8, 512]
        tok = np.ascontiguousarray(xs.T).reshape(NCH, CS * RB)  # (t, b) order
        xg = np.ascontiguousarray(tok.T.astype(np.int32))  # [64, NCH]
        in_maps.append({**shared, "xg": xg})
    return in_maps


def kernel(x, emb, Wi1, Wh1, b1, Wi2, Wh2, b2, Wd, bd):
    bias1 = bool(np.any(np.asarray(b1)))
    bias2 = bool(np.any(np.asarray(b2)))
    key = (bias1, bias2)
    if key not in _BUILT:
        _BUILT[key] = build(bias1=bias1, bias2=bias2)
    nc = _BUILT[key]
    in_maps = _prep_maps(x, emb, Wi1, Wh1, b1, Wi2, Wh2, b2, Wd, bd)
    res = run_bass_kernel_spmd(nc, in_maps, list(range(NCORES)))
    kernel.last_result = res
    y = np.concatenate([np.asarray(res.results[c]["y"], np.float32)
                        for c in range(NCORES)])
    return y
